# revision 1
# baseline (speedup 1.0000x reference)
"""3-layer GAT (4 heads x 64) + global mean pool + FC on 8 Trainium2 NeuronCores.

Strategy (graph-parallel, per the sharding hint):
  * Nodes are permuted and partitioned into 8 contiguous shards (one per core),
    degree-balanced, and within each core bin-packed into blocks of 64 dst
    nodes whose total in-degree fits in CPB chunks of 128 edge slots.
  * Per layer, each core computes the augmented projection
    h_aug = x @ [W | W@As | W@Ad] for its own node shard and the shards are
    AllGathered into a replicated DRAM table (h in bf16, per-node attention
    scores in f32).
  * Edges live on the core that owns their dst node.  Per 128-edge chunk the
    kernel gathers the per-edge src/dst scores with 16B indirect DMAs,
    computes exp(leaky_relu(ss+sd)) on the Activation engine (broadcast to
    64 columns per head), then gathers h[src] with a 512B indirect DMA that
    multiplies into the pre-filled exp() values in flight (CCE mult).
  * Segment softmax-sum becomes a matmul: a one-hot matrix O (edge -> local
    dst slot, built by is_equal against an iota) contracts the 128-edge
    chunks into PSUM, producing both the weighted numerator (256 wide) and
    the denominator (4 wide) in one accumulation group.
  * out = relu(num/den + b); residual after layer 3; graph mean-pool is a
    matmul against a host-built one-hot graph matrix followed by a tiny
    AllReduce; the final FC runs replicated on every core.
"""

import math
import os
import numpy as np

import concourse.bass as bass
import concourse.tile as tile
from concourse import bacc, mybir
from concourse.bass import IndirectOffsetOnAxis
from concourse.masks import make_identity

F32 = mybir.dt.float32
BF16 = mybir.dt.bfloat16
I32 = mybir.dt.int32
NPBF16 = mybir.dt.np(BF16)

AF = mybir.ActivationFunctionType
_SP = os.environ.get('GAT_SINGLE_PACKET', '0') == '1'
_NQ = int(os.environ.get('GAT_NQUEUES', '1'))
_DBG = int(os.environ.get('GAT_DBG', '0'))
ALU = mybir.AluOpType


class Cfg:
    def __init__(self, N=50000, E=800000, IN=64, OUT=64, H=4, G=64, C=10,
                 NCORES=8, BLOCKS=100, CPB=9, BB=2, neg_slope=0.2,
                 CPB_L=None, CPB_H=None):
        self.N, self.E, self.IN, self.OUT, self.H, self.G, self.C = N, E, IN, OUT, H, G, C
        self.HID = H * OUT                     # 256
        self.NCORES = NCORES
        self.BLOCKS = BLOCKS                   # dst blocks per core (64 nodes each)
        self.M = 64                            # dst nodes per block
        self.NLOC = BLOCKS * self.M            # nodes per core
        self.NPAD = NCORES * self.NLOC
        self.NCHUNK = self.NLOC // 128         # 128-node chunks per core
        self.PAIRS = self.NCHUNK               # block pairs per core
        self.CPB = CPB                         # kept for compat (unused)
        self.CPB_L = CPB_L if CPB_L else (CPB + 1) // 2
        self.CPB_H = CPB_H if CPB_H else (CPB + 1) // 2
        self.CPBT = self.CPB_L + self.CPB_H    # chunks per block total
        self.BB = BB                           # blocks per gather batch
        self.JB = BB * self.CPBT               # chunks per gather batch
        self.JTOT = BLOCKS * self.CPBT         # chunks per core
        self.SPLIT = self.NPAD // 2            # table half boundary (core-aligned)
        self.neg_slope = neg_slope
        assert BLOCKS % 2 == 0 and BLOCKS % BB == 0 and BB % 2 == 0
        assert self.NLOC % 128 == 0
        assert NCORES % 2 == 0
        assert self.SPLIT <= 32768 and self.NPAD - self.SPLIT <= 32768

    def chunk_map(self):
        "chunk j -> (block, start?, stop?) for the low/high grouped layout"
        out = []
        for g in range(self.BLOCKS // self.BB):
            for i in range(self.BB * self.CPB_L):
                b = g * self.BB + i // self.CPB_L
                out.append((b, i % self.CPB_L == 0, False))
            for i in range(self.BB * self.CPB_H):
                b = g * self.BB + i // self.CPB_H
                out.append((b, False, i % self.CPB_H == self.CPB_H - 1))
        return out


# ----------------------------------------------------------------------------
# Host-side preprocessing: node permutation, edge slotting, input marshalling
# ----------------------------------------------------------------------------

def _snake(n, nbins):
    """Assign n items (in priority order) to nbins bins, snake order."""
    i = np.arange(n)
    m = i % (2 * nbins)
    return np.where(m < nbins, m, 2 * nbins - 1 - m)


def preprocess(cfg, x, edge_index, batch, params):
    N = cfg.N
    NC, B, NLOC = cfg.NCORES, cfg.BLOCKS, cfg.NLOC
    src0 = np.asarray(edge_index[0], dtype=np.int64)
    dst0 = np.asarray(edge_index[1], dtype=np.int64)
    batch = np.asarray(batch, dtype=np.int64)
    x = np.asarray(x, dtype=np.float32)

    deg = np.bincount(dst0, minlength=N) + 1          # + self loop
    order = np.argsort(-deg, kind="stable")           # nodes by degree desc

    # phase A: node -> core (degree balanced snake). Table halves are
    # core-aligned: cores [0, NC/2) are the low half.
    core_of_rank = _snake(N, NC)
    core_of = np.empty(N, dtype=np.int64)
    core_of[order] = core_of_rank
    low_src = core_of < NC // 2                       # per ORIGINAL node id

    # per-node low/high in-degree (in-edges whose src is in the low half,
    # + self loop counted on the node's own side)
    dlow = np.bincount(dst0[low_src[src0]], minlength=N).astype(np.int64)
    dhigh = deg - 1 - dlow
    self_low = low_src
    dlow = dlow + self_low
    dhigh = dhigh + (~self_low)

    # phase B: node -> block within core (balance totals via snake)
    perm = np.empty(N, dtype=np.int64)
    maxlow = 0
    maxhigh = 0
    for c in range(NC):
        nodes_c = order[core_of_rank == c]            # degree-desc order
        ncn = len(nodes_c)
        assert ncn <= NLOC, (ncn, NLOC)
        blk = _snake(ncn, B)
        slot = np.zeros(ncn, dtype=np.int64)
        counts = np.zeros(B, dtype=np.int64)
        for i in range(ncn):
            b = blk[i]
            slot[i] = counts[b]
            counts[b] += 1
        assert counts.max() <= cfg.M, counts.max()
        perm[nodes_c] = c * NLOC + blk * cfg.M + slot
        maxlow = max(maxlow, int(np.bincount(blk, weights=dlow[nodes_c],
                                             minlength=B).max()))
        maxhigh = max(maxhigh, int(np.bincount(blk, weights=dhigh[nodes_c],
                                               minlength=B).max()))

    cpb_l = max(cfg.CPB_L, math.ceil(maxlow / 128))
    cpb_h = max(cfg.CPB_H, math.ceil(maxhigh / 128))
    if (cpb_l, cpb_h) != (cfg.CPB_L, cfg.CPB_H):
        cfg = Cfg(N=cfg.N, E=cfg.E, IN=cfg.IN, OUT=cfg.OUT, H=cfg.H, G=cfg.G,
                  C=cfg.C, NCORES=cfg.NCORES, BLOCKS=cfg.BLOCKS,
                  BB=cfg.BB, neg_slope=cfg.neg_slope,
                  CPB_L=cpb_l, CPB_H=cpb_h)

    # ---- edge slot construction ----
    loops = np.arange(N, dtype=np.int64)
    srcE = np.concatenate([perm[src0], perm[loops]])
    dstE = np.concatenate([perm[dst0], perm[loops]])
    lowE = np.concatenate([low_src[src0], self_low])
    # sort by (dst block asc, dst asc) with lows before highs within a block
    blkE = dstE // cfg.M                              # global block id
    key = (blkE * 2 + (~lowE)) * (cfg.NPAD + 1) + dstE
    o = np.argsort(key, kind="stable")
    srcE, dstE, lowE = srcE[o], dstE[o], lowE[o]

    # slot id within (block, half): rank within group
    grp = blkE[o] * 2 + (~lowE)                       # sorted group key
    grp_counts = np.bincount(grp, minlength=NC * B * 2)
    grp_start = np.concatenate([[0], np.cumsum(grp_counts)])[:-1]
    rank = np.arange(len(dstE)) - grp_start[grp]

    CL, CH, CT = cfg.CPB_L, cfg.CPB_H, cfg.CPBT
    BBn = cfg.BB
    # per-core slot array sizes
    nslot_core = B * CT * 128
    # compute each edge's global slot position in the grouped layout
    core_e = dstE // NLOC
    b_in_core = (dstE % NLOC) // cfg.M
    g_e = b_in_core // BBn                            # group within core
    b_in_g = b_in_core % BBn
    group_base = core_e * nslot_core + g_e * (BBn * CT * 128)
    low_slot = group_base + b_in_g * (CL * 128) + rank
    high_slot = group_base + BBn * CL * 128 + b_in_g * (CH * 128) + rank
    slot = np.where(lowE, low_slot, high_slot)
    assert rank[lowE].max() < CL * 128 and rank[~lowE].max() < CH * 128

    nslots = NC * nslot_core
    hidx_slot = np.zeros(nslots, dtype=np.int16)      # idx into table half
    sd_slot = np.zeros(nslots, dtype=np.int16)        # local dst row
    loc_slot = np.full(nslots, -1.0, dtype=np.float32)
    hidx_slot[slot] = np.where(lowE, srcE, srcE - cfg.SPLIT).astype(np.int16)
    sd_slot[slot] = (dstE % NLOC).astype(np.int16)
    loc_slot[slot] = (dstE % cfg.M).astype(np.float32)

    JT = cfg.JTOT

    def to_pj(a, c):   # [128, JTOT]: slot s -> (j=s//128, p=s%128)
        v = a[c * nslot_core:(c + 1) * nslot_core]
        return np.ascontiguousarray(v.reshape(JT, 128).T)

    # low/high gather index arrays: per group, the low (resp high) slots are
    # contiguous; concatenate per-group runs so each call slices columns.
    ngroups = B // BBn
    lowsel = np.zeros(nslot_core, dtype=bool)
    for g in range(ngroups):
        gb = g * BBn * CT * 128
        lowsel[gb:gb + BBn * CL * 128] = True

    # ---- weights ----
    W1, as1, ad1, b1 = params["W1"], params["as1"], params["ad1"], params["b1"]
    W2, as2, ad2, b2 = params["W2"], params["as2"], params["ad2"], params["b2"]
    W3, as3, ad3, b3 = params["W3"], params["as3"], params["ad3"], params["b3"]
    fcW, fcb = params["fcW"], params["fcb"]

    def aug(W, a_s, a_d):
        W = np.asarray(W, np.float32)
        HID, H, OUT = cfg.HID, cfg.H, cfg.OUT
        As = np.zeros((HID, H), np.float32)
        Ad = np.zeros((HID, H), np.float32)
        for h in range(H):
            As[h * OUT:(h + 1) * OUT, h] = np.asarray(a_s, np.float32)[h]
            Ad[h * OUT:(h + 1) * OUT, h] = np.asarray(a_d, np.float32)[h]
        return np.concatenate([W, W @ As, W @ Ad], axis=1)  # [in, HID+2H]

    w1a = aug(W1, as1, ad1)
    w2a = aug(W2, as2, ad2)
    w3a = aug(W3, as3, ad3)
    WA = cfg.HID + 2 * cfg.H                          # 264

    def pack_k(w):                                    # [256, WA] -> [128, 2*WA]
        return np.ascontiguousarray(
            w.reshape(2, 128, WA).transpose(1, 0, 2).reshape(128, 2 * WA))

    fcw_aug = np.concatenate([np.asarray(fcW, np.float32),
                              np.asarray(fcb, np.float32)[None, :]], axis=0)
    fcw_pad = np.zeros((384, cfg.C), np.float32)
    fcw_pad[:257] = fcw_aug
    fcw_m = np.ascontiguousarray(
        fcw_pad.reshape(3, 128, cfg.C).transpose(1, 0, 2).reshape(128, 3 * cfg.C))

    in_maps = []
    for c in range(NC):
        lo, hi = c * NLOC, (c + 1) * NLOC
        mask = (perm >= lo) & (perm < hi)
        origs = np.nonzero(mask)[0]
        locs = perm[origs] - lo
        xs = np.zeros((NLOC, cfg.IN), np.float32)
        xs[locs] = x[origs]
        og = np.zeros((NLOC, cfg.G), np.float32)
        og[locs, batch[origs]] = 1.0
        og_m = np.ascontiguousarray(
            og.reshape(cfg.NCHUNK, 128, cfg.G).transpose(1, 0, 2)
              .reshape(128, cfg.NCHUNK * cfg.G))
        hv = hidx_slot[c * nslot_core:(c + 1) * nslot_core]
        rep = lambda a: np.ascontiguousarray(np.tile(a.reshape(-1, 16).T, (8, 1)))
        hL = hv[lowsel]
        hH = hv[~lowsel]
        in_maps.append({
            "xT": np.ascontiguousarray(xs.T).astype(NPBF16),
            "hidxl": rep(hL),
            "hidxh": rep(hH),
            "dstloc": to_pj(loc_slot, c).astype(NPBF16),
            "og": og_m.astype(NPBF16),
            "w1": w1a.astype(NPBF16),
            "w2": pack_k(w2a).astype(NPBF16),
            "w3": pack_k(w3a).astype(NPBF16),
            "b1": np.asarray(b1, np.float32).reshape(1, cfg.HID),
            "b2": np.asarray(b2, np.float32).reshape(1, cfg.HID),
            "b3": np.asarray(b3, np.float32).reshape(1, cfg.HID),
            "fcw": fcw_m,
        })
    return cfg, in_maps


# ----------------------------------------------------------------------------
# Device program
# ----------------------------------------------------------------------------

def build_program(cfg, debug=False):
    nc = bacc.Bacc(None, target_bir_lowering=False, debug=debug,
                   num_devices=cfg.NCORES,
                   num_swdge_queues=_NQ)
    HID, WA, H, OUT = cfg.HID, cfg.HID + 2 * cfg.H, cfg.H, cfg.OUT
    NLOC, NPAD, NCHUNK, PAIRS = cfg.NLOC, cfg.NPAD, cfg.NCHUNK, cfg.PAIRS
    JB, JTOT, CPB, BB = cfg.JB, cfg.JTOT, cfg.CPB, cfg.BB
    NBATCH = cfg.BLOCKS // BB
    RG = [list(range(cfg.NCORES))]

    # I/O
    I16 = mybir.dt.int16
    CL, CH = cfg.CPB_L, cfg.CPB_H
    ROWW = 384  # padded table row (h 256 | ss 4 | sd 4 | pad)
    d_xT = nc.dram_tensor("xT", [cfg.IN, NLOC], BF16, kind="ExternalInput")
    d_hidxl = nc.dram_tensor("hidxl", [128, cfg.BLOCKS * CL * 8], I16, kind="ExternalInput")
    d_hidxh = nc.dram_tensor("hidxh", [128, cfg.BLOCKS * CH * 8], I16, kind="ExternalInput")
    d_dstloc = nc.dram_tensor("dstloc", [128, JTOT], BF16, kind="ExternalInput")
    d_og = nc.dram_tensor("og", [128, NCHUNK * cfg.G], BF16, kind="ExternalInput")
    d_w1 = nc.dram_tensor("w1", [cfg.IN, WA], BF16, kind="ExternalInput")
    d_w2 = nc.dram_tensor("w2", [128, 2 * WA], BF16, kind="ExternalInput")
    d_w3 = nc.dram_tensor("w3", [128, 2 * WA], BF16, kind="ExternalInput")
    d_b = [nc.dram_tensor(f"b{i}", [1, HID], F32, kind="ExternalInput")
           for i in (1, 2, 3)]
    d_fcw = nc.dram_tensor("fcw", [128, 3 * cfg.C], F32, kind="ExternalInput")
    d_out = nc.dram_tensor("out", [cfg.G, cfg.C], F32, kind="ExternalOutput")

    with tile.TileContext(nc, num_cores=cfg.NCORES) as tc:
        dram = tc.alloc_tile_pool(name="dram", bufs=1, space="DRAM")
        consts = tc.alloc_tile_pool(name="consts", bufs=1)
        stage = tc.alloc_tile_pool(name="stage", bufs=1)
        xtp = tc.alloc_tile_pool(name="xtp", bufs=1)
        wp = tc.alloc_tile_pool(name="wp", bufs=2)
        ep = tc.alloc_tile_pool(name="ep", bufs=2)
        pp = tc.alloc_tile_pool(name="pp", bufs=3)
        sp = tc.alloc_tile_pool(name="sp", bufs=2)
        ps_h = tc.alloc_tile_pool(name="ps_h", bufs=1, space="PSUM")
        ps_pair = tc.alloc_tile_pool(name="ps_pair", bufs=2, space="PSUM")
        ps_sd = tc.alloc_tile_pool(name="ps_sd", bufs=2, space="PSUM")
        ps_t = tc.alloc_tile_pool(name="ps_t", bufs=2, space="PSUM")
        ps_misc = tc.alloc_tile_pool(name="ps_misc", bufs=1, space="PSUM")

        # --- DRAM scratch ---
        hin_h = dram.tile([NLOC, ROWW], BF16)
        sdtbl = dram.tile([NLOC, 64], F32)
        tbls_h = [dram.tile([NPAD, ROWW], BF16, addr_space="Shared", name=f"tblh{i}")
                  for i in range(3)]
        pool_in = dram.tile([cfg.G, HID + 1], F32)
        pool_out = dram.tile([cfg.G, HID + 1], F32, addr_space="Shared")

        # --- resident constants ---
        s_hidxl = consts.tile([128, cfg.BLOCKS * CL * 8], I16)
        s_hidxh = consts.tile([128, cfg.BLOCKS * CH * 8], I16)
        s_loc = consts.tile([128, JTOT], BF16)
        s_og = consts.tile([128, NCHUNK, cfg.G], BF16)
        nc.sync.dma_start(out=s_hidxl[:], in_=d_hidxl[:, :])
        nc.sync.dma_start(out=s_hidxh[:], in_=d_hidxh[:, :])
        nc.sync.dma_start(out=s_loc[:], in_=d_dstloc[:, :])
        nc.sync.dma_start(out=s_og[:], in_=d_og[:, :].rearrange("p (i g) -> p i g", g=cfg.G))

        s_xT1 = xtp.tile([cfg.IN, NLOC], BF16, tag="xt")
        nc.sync.dma_start(out=s_xT1[:], in_=d_xT[:, :])
        s_w1 = consts.tile([cfg.IN, WA], BF16)
        nc.sync.dma_start(out=s_w1[:], in_=d_w1[:, :])
        s_w2 = consts.tile([128, 2, WA], BF16)
        nc.sync.dma_start(out=s_w2[:], in_=d_w2[:, :].rearrange("p (k w) -> p k w", k=2))
        s_w3 = consts.tile([128, 2, WA], BF16)
        nc.sync.dma_start(out=s_w3[:], in_=d_w3[:, :].rearrange("p (k w) -> p k w", k=2))
        s_fcw = consts.tile([128, 3, cfg.C], F32)
        nc.sync.dma_start(out=s_fcw[:], in_=d_fcw[:, :].rearrange("p (k c) -> p k c", k=3))

        iota32 = consts.tile([128, cfg.M], I32)
        nc.gpsimd.iota(iota32[:], pattern=[[1, cfg.M]], base=0, channel_multiplier=0)
        s_iota = consts.tile([128, cfg.M], BF16)
        nc.vector.tensor_copy(s_iota[:], iota32[:])
        ident_bf = consts.tile([128, 128], BF16)
        make_identity(nc, ident_bf[:])
        ident_f32 = consts.tile([128, 128], F32)
        make_identity(nc, ident_f32[:])
        ones_row = consts.tile([1, cfg.G], F32)
        nc.vector.memset(ones_row[:], 1.0)

        x1_res = consts.tile([128, NCHUNK, HID], BF16)   # layer-1 activations
        HC = 10 if NCHUNK % 10 == 0 else (NCHUNK // 2 if NCHUNK % 2 == 0 else NCHUNK)
        # zero-fill sdtbl once (cols 4:64 stay zero; per-layer writes touch 0:4)
        zt = consts.tile([128, 64], F32)
        nc.vector.memset(zt[:], 0.0)
        zt_b = bass.AP(tensor=zt.tensor, offset=zt[:].offset,
                       ap=[zt[:].ap[0], [0, NCHUNK], [1, 64]])
        nc.sync.dma_start(out=sdtbl[:].rearrange("(i p) w -> p i w", p=128),
                          in_=zt_b)

        sdb_ref = [None]

        def table_build(layer, xT_t, w_t, khalves):
            sd_st = stage.tile([128, NCHUNK, 4], F32, name=f"sdst{layer}", tag="sdst")
            for half in range(NCHUNK // HC):
                h_st = stage.tile([128, HC, ROWW], BF16, name=f"hst{layer}_{half}",
                                  tag="hst")
                nc.vector.memset(h_st[:, :, HID + 8:], 0.0)
                for ii in range(HC):
                    i = half * HC + ii
                    ph = ps_h.tile([128, WA], F32, name=f"ph{layer}_{i}", tag="ph")
                    for k in range(khalves):
                        if khalves == 1:
                            lhsT = xT_t[:, i * 128:(i + 1) * 128]
                            rhs = w_t[:, :]
                        else:
                            lhsT = xT_t[:, k, i * 128:(i + 1) * 128]
                            rhs = w_t[:, k, :]
                        nc.tensor.matmul(out=ph[:], lhsT=lhsT, rhs=rhs,
                                         start=(k == 0), stop=(k == khalves - 1))
                    nc.scalar.copy(out=h_st[:, ii, 0:HID], in_=ph[:, 0:HID])
                    nc.scalar.copy(out=h_st[:, ii, HID:HID + 8], in_=ph[:, HID:WA])
                    nc.vector.tensor_copy(out=sd_st[:, i, 0:4], in_=ph[:, HID + 4:WA])
                nc.sync.dma_start(
                    out=hin_h[half * HC * 128:(half + 1) * HC * 128, :]
                        .rearrange("(i p) w -> p i w", p=128),
                    in_=h_st[:])
            nc.sync.dma_start(out=sdtbl[:, 0:4].rearrange("(i p) w -> p i w", p=128),
                              in_=sd_st[:])
            sdb = wp.tile([64, cfg.BLOCKS, 4], F32, name=f"sdb{layer}", tag="sdb")
            nc.sync.dma_start(
                out=sdb[:],
                in_=sdtbl[:, 0:4].rearrange("(b m) w -> m b w", m=cfg.M))
            sdb_ref[0] = sdb
            nc.gpsimd.collective_compute(
                "AllGather", ALU.bypass, replica_groups=RG,
                ins=[hin_h[:].opt()], outs=[tbls_h[layer][:].opt()])

        def edge_phase(layer, bias_t, xT_next, ps_pool_t):
            tbl_h = tbls_h[layer]
            cmap = cfg.chunk_map()
            blk_psum = {}
            NBATCH = cfg.BLOCKS // BB
            nL, nH = BB * CL * 128, BB * CH * 128
            for g in range(NBATCH):
                j0 = g * JB
                gt = ep.tile([128, JB, ROWW], BF16, name=f"gt{layer}_{g}", tag="gt", bufs=3)
                nc.gpsimd.dma_gather(
                    out_ap=gt[:, 0:BB * CL, :], in_ap=tbl_h[0:cfg.SPLIT, :],
                    idxs_ap=s_hidxl[:, g * (nL // 16):(g + 1) * (nL // 16)],
                    num_idxs=nL, num_idxs_reg=nL, elem_size=ROWW,
                    single_packet=_SP, queue_num=0)
                nc.gpsimd.dma_gather(
                    out_ap=gt[:, BB * CL:JB, :], in_ap=tbl_h[cfg.SPLIT:NPAD, :],
                    idxs_ap=s_hidxh[:, g * (nH // 16):(g + 1) * (nH // 16)],
                    num_idxs=nH, num_idxs_reg=nH, elem_size=ROWW,
                    single_packet=_SP, queue_num=1 % _NQ)
                o_t = ep.tile([128, JB, cfg.M], BF16, name=f"o{layer}_{g}", tag="o", bufs=3)
                iota_b = bass.AP(tensor=s_iota.tensor, offset=s_iota[:].offset,
                                 ap=[s_iota[:].ap[0], [0, JB], s_iota[:].ap[1]])
                nc.vector.tensor_tensor(
                    out=o_t[:], in0=iota_b,
                    in1=s_loc[:, j0:j0 + JB].to_broadcast([128, JB, cfg.M]),
                    op=ALU.is_equal)
                sdb = sdb_ref[0]
                ot_sb = ep.tile([64, JB, 128], F32, name=f"ot{layer}_{g}", tag="ot")
                for cp in range(JB // 2):
                    ptt = ps_t.tile([128, 128], BF16, name=f"ott{layer}_{g}_{cp}", tag="pt")
                    nc.tensor.transpose(
                        out=ptt[:],
                        in_=o_t[:, 2 * cp:2 * cp + 2, :].rearrange("p a m -> p (a m)"),
                        identity=ident_bf[:])
                    nc.scalar.copy(out=ot_sb[:, 2 * cp, :], in_=ptt[0:64, :])
                    nc.scalar.copy(out=ot_sb[:, 2 * cp + 1, :], in_=ptt[64:128, :])
                sdps = ps_sd.tile([128, JB, H], F32, name=f"sdps{layer}_{g}", tag="sdps")
                for c in range(JB):
                    b = cmap[j0 + c][0]
                    nc.tensor.matmul(
                        out=sdps[:, c, :],
                        lhsT=ot_sb[:, c, :],
                        rhs=sdb[:, b, :],
                        start=True, stop=True)
                e_t = ep.tile([128, JB, H], F32, name=f"e{layer}_{g}", tag="e")
                nc.vector.tensor_tensor(out=e_t[:], in0=gt[:, :, HID:HID + H],
                                        in1=sdps[:], op=ALU.add)
                el = ep.tile([128, JB, H], F32, name=f"el{layer}_{g}", tag="el")
                nc.vector.tensor_scalar(out=el[:], in0=e_t[:], scalar1=cfg.neg_slope,
                                        scalar2=None, op0=ALU.mult)
                nc.vector.tensor_tensor(out=e_t[:], in0=e_t[:], in1=el[:], op=ALU.max)
                nc.scalar.activation(out=gt[:, :, HID:HID + H], in_=e_t[:], func=AF.Exp)
                nc.vector.tensor_tensor(
                    out=gt[:, :, 0:HID].rearrange("p a (h o) -> p a h o", o=OUT),
                    in0=gt[:, :, 0:HID].rearrange("p a (h o) -> p a h o", o=OUT),
                    in1=gt[:, :, HID:HID + H].to_broadcast([128, JB, H, OUT]),
                    op=ALU.mult)
                for c in range(JB):
                    j = j0 + c
                    b, is_start, is_stop = cmap[j]
                    if is_start:
                        blk_psum[b] = ps_pair.tile(
                            [64, WA - H], F32, name=f"pp{layer}_{b}", tag="pp")
                    nc.tensor.matmul(
                        out=blk_psum[b][:], lhsT=o_t[:, c, :],
                        rhs=gt[:, c, 0:WA - H],
                        start=is_start, stop=is_stop)
                    if is_stop and b % 2 == 1:
                        pair = b // 2
                        postproc(layer, pair, blk_psum[b - 1], blk_psum[b],
                                 bias_t, xT_next, ps_pool_t)
                        del blk_psum[b - 1], blk_psum[b]

        def postproc(layer, pair, ppz0, ppz1, bias_t, xT_next, ps_pool_t):
            den = pp.tile([128, H], F32, name=f"den{layer}_{pair}", tag="den")
            for hf, ppz in ((0, ppz0), (1, ppz1)):
                nc.vector.tensor_scalar(out=den[hf * 64:(hf + 1) * 64, :],
                                        in0=ppz[:, HID:HID + H],
                                        scalar1=1e-30, scalar2=None, op0=ALU.max)
            nc.vector.reciprocal(den[:], den[:])
            xf = pp.tile([128, HID], F32, name=f"xf{layer}_{pair}", tag="xf")
            for hf, ppz in ((0, ppz0), (1, ppz1)):
                nc.vector.tensor_tensor(
                    out=xf[hf * 64:(hf + 1) * 64, :].rearrange("p (h o) -> p h o", o=OUT),
                    in0=ppz[:, 0:HID].rearrange("p (h o) -> p h o", o=OUT),
                    in1=den[hf * 64:(hf + 1) * 64, :].to_broadcast([64, H, OUT]),
                    op=ALU.mult)
            nc.vector.tensor_tensor(out=xf[:], in0=xf[:], in1=bias_t[:], op=ALU.add)
            if layer == 0:
                xb = x1_res[:, pair, :]
            else:
                xb = pp.tile([128, HID], BF16, name=f"xb{layer}_{pair}", tag="xb")
            nc.scalar.activation(out=xb, in_=xf[:], func=AF.Relu)
            if layer < 2:
                for k in (0, 1):
                    pt = ps_t.tile([128, 128], BF16, name=f"pt{layer}_{pair}_{k}", tag="pt")
                    nc.tensor.transpose(out=pt[:], in_=xb[:, k * 128:(k + 1) * 128],
                                        identity=ident_bf[:])
                    nc.vector.tensor_copy(
                        out=xT_next[:, k, pair * 128:(pair + 1) * 128], in_=pt[:])
            else:
                xr = pp.tile([128, HID + 1], BF16, name=f"xr{pair}", tag="xr")
                nc.vector.memset(xr[:, HID:HID + 1], 1.0)
                nc.vector.tensor_tensor(out=xr[:, 0:HID], in0=xb,
                                        in1=x1_res[:, pair, :], op=ALU.add)
                nc.tensor.matmul(out=ps_pool_t[:], lhsT=s_og[:, pair, :],
                                 rhs=xr[:], start=(pair == 0),
                                 stop=(pair == PAIRS - 1))

        # ---------------- main flow ----------------
        bias_ts = []
        for i in range(3):
            bt = wp.tile([128, HID], F32, name=f"bias{i}", tag="bias")
            nc.sync.dma_start(out=bt[:], in_=bass.AP(
                tensor=d_b[i][:, :].tensor, offset=0, ap=[[0, 128], [1, HID]]))
            bias_ts.append(bt)

        ps_pool_t = ps_misc.tile([cfg.G, HID + 1], F32, tag="misc")

        table_build(0, s_xT1, s_w1, 1)
        xT2 = xtp.tile([128, 2, NLOC], BF16, name="xT2", tag="xt")
        edge_phase(0, bias_ts[0], xT2, None)

        table_build(1, xT2, s_w2, 2)
        xT3 = xtp.tile([128, 2, NLOC], BF16, name="xT3", tag="xt")
        edge_phase(1, bias_ts[1], xT3, None)

        table_build(2, xT3, s_w3, 2)
        edge_phase(2, bias_ts[2], None, ps_pool_t)

        # ---------------- epilogue ----------------
        pl = sp.tile([cfg.G, HID + 1], F32)
        nc.vector.tensor_copy(pl[:], ps_pool_t[:])
        nc.sync.dma_start(out=pool_in[:, :], in_=pl[:])
        nc.gpsimd.collective_compute(
            "AllReduce", ALU.add, replica_groups=RG,
            ins=[pool_in[:].opt()], outs=[pool_out[:].opt()])
        pr = sp.tile([cfg.G, HID + 1], F32)
        nc.sync.dma_start(out=pr[:], in_=pool_out[:, :])
        cnt = sp.tile([cfg.G, 1], F32)
        nc.vector.tensor_scalar(out=cnt[:], in0=pr[:, HID:HID + 1],
                                scalar1=1.0, scalar2=None, op0=ALU.max)
        nc.vector.reciprocal(cnt[:], cnt[:])
        pa = sp.tile([cfg.G, HID + 1], F32)
        nc.vector.tensor_scalar(out=pa[:, 0:HID], in0=pr[:, 0:HID],
                                scalar1=cnt[:, 0:1], scalar2=None, op0=ALU.mult)
        nc.vector.memset(pa[:, HID:HID + 1], 1.0)
        paT = sp.tile([128, 2, cfg.G], F32)
        for k in (0, 1):
            pt = ps_t.tile([128, 128], F32, name=f"ptfc{k}", tag="pt")
            nc.tensor.transpose(out=pt[:, 0:cfg.G],
                                in_=pa[:, k * 128:(k + 1) * 128],
                                identity=ident_f32[0:cfg.G, 0:cfg.G])
            nc.vector.tensor_copy(paT[:, k, :], pt[:, 0:cfg.G])
        pfc = ps_misc.tile([cfg.G, cfg.C], F32, tag="misc")
        nc.tensor.matmul(out=pfc[:], lhsT=paT[:, 0, :], rhs=s_fcw[:, 0, :],
                         start=True, stop=False)
        nc.tensor.matmul(out=pfc[:], lhsT=paT[:, 1, :], rhs=s_fcw[:, 1, :],
                         start=False, stop=False)
        nc.tensor.matmul(out=pfc[:], lhsT=ones_row[:], rhs=s_fcw[0:1, 2, :],
                         start=False, stop=True)
        outt = sp.tile([cfg.G, cfg.C], F32)
        nc.vector.tensor_copy(outt[:], pfc[:])
        nc.sync.dma_start(out=d_out[:, :], in_=outt[:])

        for _pool in (ps_misc, ps_t, ps_sd, ps_pair, ps_h, sp, pp, ep, wp, xtp,
                      stage, consts, dram):
            _pool.release()

    nc.compile()
    return nc


# ----------------------------------------------------------------------------
# Entry point
# ----------------------------------------------------------------------------

_CACHE = {}


def _get_program(cfg):
    key = (cfg.N, cfg.BLOCKS, cfg.CPB, cfg.BB, cfg.NCORES)
    if key not in _CACHE:
        _CACHE[key] = build_program(cfg)
    return _CACHE[key]


def kernel(x, edge_index, batch, W1, as1, ad1, b1, W2, as2, ad2, b2,
           W3, as3, ad3, b3, fcW, fcb):
    from concourse.bass_utils import run_bass_kernel_spmd
    cfg = Cfg()
    params = dict(W1=W1, as1=as1, ad1=ad1, b1=b1, W2=W2, as2=as2, ad2=ad2,
                  b2=b2, W3=W3, as3=as3, ad3=ad3, b3=b3, fcW=fcW, fcb=fcb)
    cfg, in_maps = preprocess(cfg, x, edge_index, batch, params)
    nc = _get_program(cfg)
    res = run_bass_kernel_spmd(nc, in_maps, core_ids=list(range(cfg.NCORES)))
    return np.asarray(res.results[0]["out"], dtype=np.float32)



# revision 5
# speedup vs baseline: 1.3469x; 1.3469x over previous
"""3-layer GAT (4 heads x 64) + global mean pool + FC on 8 Trainium2 NeuronCores.

Strategy (graph-parallel):
  * Nodes are permuted and partitioned into 8 contiguous shards (one per core),
    degree-balanced, and within each core greedily bin-packed into 100 blocks
    of 64 dst nodes balancing both low- and high-half in-degree.
  * Per layer, each core computes the augmented projection
    h_aug = x @ [W | W@As | W@Ad] for its own node shard; shards are
    AllGathered into a replicated DRAM table (row = 256 h bf16 + 4 ss + pad).
  * Edges live on the core that owns their dst node, sorted by (block, src
    half, dst).  The table is split at row SPLIT (3.5 cores ~ 44% of edge
    mass) so int16 gather indices reach both halves; per-block slot budget is
    4 low + 5 high 128-slot chunks.
  * Per 128-edge chunk the kernel gathers 768B table rows with SWDGE indirect
    DMAs round-robined over 4 queues (the per-queue descriptor feed is the
    bottleneck), adds the dst attention score via a one-hot matmul
    (host-precomputed dst-major one-hot x per-chunk sd from SBUF), applies
    leaky-relu+exp on the Activation engine, multiplies h by the per-edge
    weight, and contracts numerator+denominator into PSUM with a
    host-precomputed slot-major one-hot.
  * out = relu(num/den + b); residual after layer 3; the next layer's
    projection chunk is emitted inline right after each block pair finishes
    so only the AllGather sits between layers.  Graph mean-pool is a matmul
    against a host-built one-hot graph matrix + a tiny AllReduce; the final
    FC runs replicated on every core.
"""

import math
import os
import numpy as np

import concourse.bass as bass
import concourse.tile as tile
from concourse import bacc, mybir
from concourse.masks import make_identity

F32 = mybir.dt.float32
BF16 = mybir.dt.bfloat16
I32 = mybir.dt.int32
I16 = mybir.dt.int16
NPBF16 = mybir.dt.np(BF16)

AF = mybir.ActivationFunctionType
ALU = mybir.AluOpType
_NQ = int(os.environ.get('GAT_NQUEUES', '4'))
_SP = os.environ.get('GAT_SINGLE_PACKET', '0') == '1'


class Cfg:
    def __init__(self, N=50000, E=800000, IN=64, OUT=64, H=4, G=64, C=10,
                 NCORES=8, BLOCKS=100, BB=2, neg_slope=0.2,
                 CPB_L=4, CPB_H=5, SPLIT_BLOCKS=350):
        self.N, self.E, self.IN, self.OUT, self.H, self.G, self.C = N, E, IN, OUT, H, G, C
        self.HID = H * OUT                     # 256
        self.NCORES = NCORES
        self.BLOCKS = BLOCKS                   # dst blocks per core (64 nodes each)
        self.M = 64                            # dst nodes per block
        self.NLOC = BLOCKS * self.M            # nodes per core
        self.NPAD = NCORES * self.NLOC
        self.NCHUNK = self.NLOC // 128         # 128-node chunks per core
        self.PAIRS = self.NCHUNK
        self.CPB_L = CPB_L
        self.CPB_H = CPB_H
        self.CPBT = CPB_L + CPB_H              # chunks per block total
        self.BB = BB                           # blocks per batch (= node chunk)
        self.JB = BB * self.CPBT               # chunks per batch
        self.JTOT = BLOCKS * self.CPBT         # chunks per core
        self.NBATCH = BLOCKS // BB
        # table half boundary, in blocks across the whole table
        self.SPLIT_BLOCKS = SPLIT_BLOCKS
        self.SPLIT = SPLIT_BLOCKS * self.M
        self.neg_slope = neg_slope
        assert BLOCKS % 2 == 0 and BB == 2
        assert self.NLOC % 128 == 0
        assert self.SPLIT % self.NLOC in (0, self.NLOC // 2)  # core or half-core aligned
        assert self.SPLIT <= 32768 and self.NPAD - self.SPLIT <= 32768

    def chunk_map(self):
        "chunk j -> (block, start?, stop?) for the low/high grouped layout"
        out = []
        for g in range(self.NBATCH):
            for i in range(self.BB * self.CPB_L):
                b = g * self.BB + i // self.CPB_L
                out.append((b, i % self.CPB_L == 0, False))
            for i in range(self.BB * self.CPB_H):
                b = g * self.BB + i // self.CPB_H
                out.append((b, False, i % self.CPB_H == self.CPB_H - 1))
        return out


# ----------------------------------------------------------------------------
# Host-side preprocessing
# ----------------------------------------------------------------------------

def _snake(n, nbins):
    i = np.arange(n)
    m = i % (2 * nbins)
    return np.where(m < nbins, m, 2 * nbins - 1 - m)


def _pack_blocks(dlo, dhi, nblocks, cap_lo, cap_hi, M):
    """Greedily assign nodes (with per-node low/high in-degree) to nblocks
    blocks of at most M nodes, balancing both dims against the caps."""
    order = np.argsort(-(dlo + dhi), kind="stable")
    bl = np.zeros(nblocks)
    bh = np.zeros(nblocks)
    cnt = np.zeros(nblocks, np.int64)
    blk = np.empty(len(dlo), np.int64)
    for n in order:
        score = np.maximum((bl + dlo[n]) / cap_lo, (bh + dhi[n]) / cap_hi)
        score[cnt >= M] = np.inf
        b = int(np.argmin(score))
        blk[n] = b
        bl[b] += dlo[n]
        bh[b] += dhi[n]
        cnt[b] += 1
    return blk, int(bl.max()), int(bh.max())


def preprocess(cfg, x, edge_index, batch, params):
    N = cfg.N
    NC, B, NLOC, M = cfg.NCORES, cfg.BLOCKS, cfg.NLOC, cfg.M
    src0 = np.asarray(edge_index[0], dtype=np.int64)
    dst0 = np.asarray(edge_index[1], dtype=np.int64)
    batch = np.asarray(batch, dtype=np.int64)
    x = np.asarray(x, dtype=np.float32)

    deg = np.bincount(dst0, minlength=N) + 1          # + self loop
    order = np.argsort(-deg, kind="stable")           # nodes by in-degree desc

    # phase A: node -> core (degree-balanced snake)
    core_of_rank = _snake(N, NC)
    core_of = np.empty(N, dtype=np.int64)
    core_of[order] = core_of_rank

    # low/high classification: rows < SPLIT are "low".  SPLIT sits at
    # SPLIT_BLOCKS blocks; full cores below it are all-low, the boundary
    # core is split in half (its nodes snake-split into the two halves).
    split_core = cfg.SPLIT // NLOC                     # first (possibly) split core
    split_mid = (cfg.SPLIT % NLOC) != 0
    low_node = core_of < split_core
    half_lo_of = np.zeros(N, dtype=bool)               # for the split core only
    if split_mid:
        nodes_sc = order[core_of_rank == split_core]   # degree-desc order
        sel = (np.arange(len(nodes_sc)) % 2) == 0      # alternate halves
        half_lo_of[nodes_sc[sel]] = True
        low_node |= (core_of == split_core) & half_lo_of

    # per-node low/high in-degree
    dlow = np.bincount(dst0[low_node[src0]], minlength=N).astype(np.int64)
    dhigh = deg - 1 - dlow
    dlow = dlow + low_node
    dhigh = dhigh + (~low_node)

    # phase B: node -> block within core, 2D-balanced greedy packing
    CAP_L, CAP_H = cfg.CPB_L * 128, cfg.CPB_H * 128
    perm = np.empty(N, dtype=np.int64)
    maxlow = 0
    maxhigh = 0
    for c in range(NC):
        nodes_c = order[core_of_rank == c]
        if split_mid and c == split_core:
            parts = [(nodes_c[half_lo_of[nodes_c]], 0, B // 2),
                     (nodes_c[~half_lo_of[nodes_c]], B // 2, B // 2)]
        else:
            parts = [(nodes_c, 0, B)]
        for nodes_p, b0, nb in parts:
            assert len(nodes_p) <= nb * M
            blk, ml, mh = _pack_blocks(dlow[nodes_p], dhigh[nodes_p],
                                       nb, CAP_L, CAP_H, M)
            maxlow = max(maxlow, ml)
            maxhigh = max(maxhigh, mh)
            slot = np.zeros(len(nodes_p), dtype=np.int64)
            counts = np.zeros(nb, dtype=np.int64)
            for i in range(len(nodes_p)):
                b = blk[i]
                slot[i] = counts[b]
                counts[b] += 1
            perm[nodes_p] = c * NLOC + (b0 + blk) * M + slot

    cpb_l = max(cfg.CPB_L, math.ceil(maxlow / 128))
    cpb_h = max(cfg.CPB_H, math.ceil(maxhigh / 128))
    if (cpb_l, cpb_h) != (cfg.CPB_L, cfg.CPB_H):
        cfg = Cfg(N=cfg.N, E=cfg.E, IN=cfg.IN, OUT=cfg.OUT, H=cfg.H, G=cfg.G,
                  C=cfg.C, NCORES=cfg.NCORES, BLOCKS=cfg.BLOCKS, BB=cfg.BB,
                  neg_slope=cfg.neg_slope, CPB_L=cpb_l, CPB_H=cpb_h,
                  SPLIT_BLOCKS=cfg.SPLIT_BLOCKS)

    # ---- edge slot construction ----
    loops = np.arange(N, dtype=np.int64)
    srcE = np.concatenate([perm[src0], perm[loops]])
    dstE = np.concatenate([perm[dst0], perm[loops]])
    lowE = srcE < cfg.SPLIT
    blkE = dstE // M
    key = (blkE * 2 + (~lowE)) * (cfg.NPAD + 1) + dstE
    o = np.argsort(key, kind="stable")
    srcE, dstE, lowE = srcE[o], dstE[o], lowE[o]

    grp = blkE[o] * 2 + (~lowE)
    grp_counts = np.bincount(grp, minlength=NC * B * 2)
    grp_start = np.concatenate([[0], np.cumsum(grp_counts)])[:-1]
    rank = np.arange(len(dstE)) - grp_start[grp]

    CL, CH, CT = cfg.CPB_L, cfg.CPB_H, cfg.CPBT
    BBn = cfg.BB
    nslot_core = B * CT * 128
    core_e = dstE // NLOC
    b_in_core = (dstE % NLOC) // M
    g_e = b_in_core // BBn
    b_in_g = b_in_core % BBn
    group_base = core_e * nslot_core + g_e * (BBn * CT * 128)
    low_slot = group_base + b_in_g * (CL * 128) + rank
    high_slot = group_base + BBn * CL * 128 + b_in_g * (CH * 128) + rank
    slot = np.where(lowE, low_slot, high_slot)
    assert rank[lowE].max() < CL * 128 and rank[~lowE].max() < CH * 128

    nslots = NC * nslot_core
    hidx_slot = np.zeros(nslots, dtype=np.int16)
    dloc_slot = np.full(nslots, -1, dtype=np.int64)    # dst row within core
    hidx_slot[slot] = np.where(lowE, srcE, srcE - cfg.SPLIT).astype(np.int16)
    dloc_slot[slot] = dstE % NLOC

    JT = cfg.JTOT
    ngroups = B // BBn
    lowsel = np.zeros(nslot_core, dtype=bool)
    for g in range(ngroups):
        gb = g * BBn * CT * 128
        lowsel[gb:gb + BBn * CL * 128] = True

    # ---- weights ----
    W1, as1, ad1, b1 = params["W1"], params["as1"], params["ad1"], params["b1"]
    W2, as2, ad2, b2 = params["W2"], params["as2"], params["ad2"], params["b2"]
    W3, as3, ad3, b3 = params["W3"], params["as3"], params["ad3"], params["b3"]
    fcW, fcb = params["fcW"], params["fcb"]

    def aug(W, a_s, a_d):
        W = np.asarray(W, np.float32)
        HID, H, OUT = cfg.HID, cfg.H, cfg.OUT
        As = np.zeros((HID, H), np.float32)
        Ad = np.zeros((HID, H), np.float32)
        for h in range(H):
            As[h * OUT:(h + 1) * OUT, h] = np.asarray(a_s, np.float32)[h]
            Ad[h * OUT:(h + 1) * OUT, h] = np.asarray(a_d, np.float32)[h]
        return np.concatenate([W, W @ As, W @ Ad], axis=1)  # [in, HID+2H]

    w1a = aug(W1, as1, ad1)
    w2a = aug(W2, as2, ad2)
    w3a = aug(W3, as3, ad3)
    WA = cfg.HID + 2 * cfg.H                          # 264

    def pack_k(w):                                    # [256, WA] -> [128, 2*WA]
        return np.ascontiguousarray(
            w.reshape(2, 128, WA).transpose(1, 0, 2).reshape(128, 2 * WA))

    fcw_aug = np.concatenate([np.asarray(fcW, np.float32),
                              np.asarray(fcb, np.float32)[None, :]], axis=0)
    fcw_pad = np.zeros((384, cfg.C), np.float32)
    fcw_pad[:257] = fcw_aug
    fcw_m = np.ascontiguousarray(
        fcw_pad.reshape(3, 128, cfg.C).transpose(1, 0, 2).reshape(128, 3 * cfg.C))

    in_maps = []
    for c in range(NC):
        lo, hi = c * NLOC, (c + 1) * NLOC
        mask = (perm >= lo) & (perm < hi)
        origs = np.nonzero(mask)[0]
        locs = perm[origs] - lo
        xs = np.zeros((NLOC, cfg.IN), np.float32)
        xs[locs] = x[origs]
        og = np.zeros((NLOC, cfg.G), np.float32)
        og[locs, batch[origs]] = 1.0
        og_m = np.ascontiguousarray(
            og.reshape(cfg.NCHUNK, 128, cfg.G).transpose(1, 0, 2)
              .reshape(128, cfg.NCHUNK * cfg.G))
        hv = hidx_slot[c * nslot_core:(c + 1) * nslot_core]
        rep = lambda a: np.ascontiguousarray(np.tile(a.reshape(-1, 16).T, (8, 1)))
        # one-hots from the slot -> dst-row map
        dl = dloc_slot[c * nslot_core:(c + 1) * nslot_core]
        jj = np.arange(nslot_core) // 128              # chunk of each slot
        pp = np.arange(nslot_core) % 128               # partition of each slot
        valid = dl >= 0
        # slot-major [128, JTOT*64]: (p, j*64 + dst%64)
        o_ag = np.zeros((128, JT * 64), dtype=NPBF16)
        o_ag[pp[valid], jj[valid] * 64 + (dl[valid] % 64)] = 1.0
        # dst-major [128, JTOT*128]: (dst%128, j*128 + p)
        o_sd = np.zeros((128, JT * 128), dtype=NPBF16)
        o_sd[dl[valid] % 128, jj[valid] * 128 + pp[valid]] = 1.0
        in_maps.append({
            "xT": np.ascontiguousarray(xs.T).astype(NPBF16),
            "hidxl": rep(hv[lowsel]),
            "hidxh": rep(hv[~lowsel]),
            "oag": o_ag,
            "osd": o_sd,
            "og": og_m.astype(NPBF16),
            "w1": w1a.astype(NPBF16),
            "w2": pack_k(w2a).astype(NPBF16),
            "w3": pack_k(w3a).astype(NPBF16),
            "b1": np.asarray(b1, np.float32).reshape(1, cfg.HID),
            "b2": np.asarray(b2, np.float32).reshape(1, cfg.HID),
            "b3": np.asarray(b3, np.float32).reshape(1, cfg.HID),
            "fcw": fcw_m,
        })
    return cfg, in_maps


# ----------------------------------------------------------------------------
# Device program
# ----------------------------------------------------------------------------

def build_program(cfg, debug=False):
    nc = bacc.Bacc(None, target_bir_lowering=False, debug=debug,
                   num_devices=cfg.NCORES, num_swdge_queues=_NQ)
    HID, WA, H, OUT = cfg.HID, cfg.HID + 2 * cfg.H, cfg.H, cfg.OUT
    NLOC, NPAD, NCHUNK, PAIRS = cfg.NLOC, cfg.NPAD, cfg.NCHUNK, cfg.PAIRS
    JB, JTOT = cfg.JB, cfg.JTOT
    NBATCH = cfg.NBATCH
    CL, CH = cfg.CPB_L, cfg.CPB_H
    RG = [list(range(cfg.NCORES))]
    ROWW = 384  # padded table row (h 256 | ss 4 | sd 4 | pad)

    d_xT = nc.dram_tensor("xT", [cfg.IN, NLOC], BF16, kind="ExternalInput")
    d_hidxl = nc.dram_tensor("hidxl", [128, cfg.BLOCKS * CL * 8], I16, kind="ExternalInput")
    d_hidxh = nc.dram_tensor("hidxh", [128, cfg.BLOCKS * CH * 8], I16, kind="ExternalInput")
    d_oag = nc.dram_tensor("oag", [128, JTOT * 64], BF16, kind="ExternalInput")
    d_osd = nc.dram_tensor("osd", [128, JTOT * 128], BF16, kind="ExternalInput")
    d_og = nc.dram_tensor("og", [128, NCHUNK * cfg.G], BF16, kind="ExternalInput")
    d_w1 = nc.dram_tensor("w1", [cfg.IN, WA], BF16, kind="ExternalInput")
    d_w2 = nc.dram_tensor("w2", [128, 2 * WA], BF16, kind="ExternalInput")
    d_w3 = nc.dram_tensor("w3", [128, 2 * WA], BF16, kind="ExternalInput")
    d_b = [nc.dram_tensor(f"b{i}", [1, HID], F32, kind="ExternalInput")
           for i in (1, 2, 3)]
    d_fcw = nc.dram_tensor("fcw", [128, 3 * cfg.C], F32, kind="ExternalInput")
    d_out = nc.dram_tensor("out", [cfg.G, cfg.C], F32, kind="ExternalOutput")

    with tile.TileContext(nc, num_cores=cfg.NCORES) as tc:
        dram = tc.alloc_tile_pool(name="dram", bufs=1, space="DRAM")
        consts = tc.alloc_tile_pool(name="consts", bufs=1)
        stage = tc.alloc_tile_pool(name="stage", bufs=3)
        xtp = tc.alloc_tile_pool(name="xtp", bufs=1)
        wp = tc.alloc_tile_pool(name="wp", bufs=2)
        ep = tc.alloc_tile_pool(name="ep", bufs=3)
        pp = tc.alloc_tile_pool(name="pp", bufs=3)
        sp = tc.alloc_tile_pool(name="sp", bufs=2)
        ps_h = tc.alloc_tile_pool(name="ps_h", bufs=1, space="PSUM")
        ps_pair = tc.alloc_tile_pool(name="ps_pair", bufs=2, space="PSUM")
        ps_sd = tc.alloc_tile_pool(name="ps_sd", bufs=2, space="PSUM")
        ps_t = tc.alloc_tile_pool(name="ps_t", bufs=2, space="PSUM")
        ps_misc = tc.alloc_tile_pool(name="ps_misc", bufs=1, space="PSUM")

        # --- DRAM scratch ---
        hin_h = dram.tile([NLOC, ROWW], BF16)
        # one pad row: gathers of the last row read 768B from a 528B-used row
        tbls_h = [dram.tile([NPAD + 1, ROWW], BF16, addr_space="Shared",
                            name=f"tblh{i}") for i in range(3)]
        pool_in = dram.tile([cfg.G, HID + 1], F32)
        pool_out = dram.tile([cfg.G, HID + 1], F32, addr_space="Shared")

        # --- resident constants ---
        s_hidxl = consts.tile([128, cfg.BLOCKS * CL * 8], I16)
        s_hidxh = consts.tile([128, cfg.BLOCKS * CH * 8], I16)
        s_og = consts.tile([128, NCHUNK, cfg.G], BF16)
        nc.sync.dma_start(out=s_hidxl[:], in_=d_hidxl[:, :])
        nc.sync.dma_start(out=s_hidxh[:], in_=d_hidxh[:, :])
        nc.sync.dma_start(out=s_og[:], in_=d_og[:, :].rearrange("p (i g) -> p i g", g=cfg.G))

        s_xT1 = xtp.tile([cfg.IN, NLOC], BF16, tag="xt")
        nc.sync.dma_start(out=s_xT1[:], in_=d_xT[:, :])
        s_w1 = consts.tile([cfg.IN, WA], BF16)
        nc.sync.dma_start(out=s_w1[:], in_=d_w1[:, :])
        s_w2 = consts.tile([128, 2, WA], BF16)
        nc.sync.dma_start(out=s_w2[:], in_=d_w2[:, :].rearrange("p (k w) -> p k w", k=2))
        s_w3 = consts.tile([128, 2, WA], BF16)
        nc.sync.dma_start(out=s_w3[:], in_=d_w3[:, :].rearrange("p (k w) -> p k w", k=2))
        s_fcw = consts.tile([128, 3, cfg.C], F32)
        nc.sync.dma_start(out=s_fcw[:], in_=d_fcw[:, :].rearrange("p (k c) -> p k c", k=3))

        ident_bf = consts.tile([128, 128], BF16)
        make_identity(nc, ident_bf[:])
        ident_f32 = consts.tile([128, 128], F32)
        make_identity(nc, ident_f32[:])
        ones_row = consts.tile([1, cfg.G], F32)
        nc.vector.memset(ones_row[:], 1.0)

        x1_res = consts.tile([128, NCHUNK, HID], BF16)   # layer-1 activations

        bias_ts = []
        for i in range(3):
            bt = consts.tile([128, HID], F32, name=f"bias{i}", tag=f"bias{i}")
            nc.sync.dma_start(out=bt[:], in_=bass.AP(
                tensor=d_b[i][:, :].tensor, offset=0, ap=[[0, 128], [1, HID]]))
            bias_ts.append(bt)

        sdb_ref = [None, None]  # sdb tiles for current / next layer

        def table_chunk(layer, i, xT_t, w_t, khalves, sdb):
            """Emit projection of node-chunk i into table `layer` + sd capture."""
            ph = ps_h.tile([128, WA], F32, name=f"ph{layer}_{i}", tag="ph")
            for k in range(khalves):
                if khalves == 1:
                    lhsT = xT_t[:, i * 128:(i + 1) * 128]
                    rhs = w_t[:, :]
                else:
                    lhsT = xT_t[:, k, i * 128:(i + 1) * 128]
                    rhs = w_t[:, k, :]
                nc.tensor.matmul(out=ph[:], lhsT=lhsT, rhs=rhs,
                                 start=(k == 0), stop=(k == khalves - 1))
            h_st = stage.tile([128, HID + H], BF16, name=f"hst{layer}_{i}", tag="hst")
            nc.scalar.copy(out=h_st[:], in_=ph[:, 0:HID + H])
            nc.vector.tensor_copy(out=sdb[:, i, :], in_=ph[:, HID + H:WA])
            nc.sync.dma_start(
                out=hin_h[i * 128:(i + 1) * 128, 0:HID + H], in_=h_st[:])

        def edge_phase(layer, bias_t, xT_next, w_next, ps_pool_t):
            tbl_h = tbls_h[layer]
            cmap = cfg.chunk_map()
            sdb = sdb_ref[0]
            sdb_next = sdb_ref[1]
            nL, nH = cfg.BB * CL * 128, cfg.BB * CH * 128
            blk_psum = {}
            for g in range(NBATCH):
                j0 = g * JB
                # one-hot loads (independent of the table -> prefetch freely)
                oag = ep.tile([128, JB, 64], BF16, name=f"oag{layer}_{g}", tag="oag")
                nc.sync.dma_start(out=oag[:], in_=d_oag[:, j0 * 64:(j0 + JB) * 64]
                                  .rearrange("p (j m) -> p j m", m=64))
                osd = ep.tile([128, JB, 128], BF16, name=f"osd{layer}_{g}", tag="osd")
                nc.sync.dma_start(out=osd[:], in_=d_osd[:, j0 * 128:(j0 + JB) * 128]
                                  .rearrange("p (j m) -> p j m", m=128))
                gt = ep.tile([128, JB, ROWW], BF16, name=f"gt{layer}_{g}", tag="gt")
                nc.gpsimd.dma_gather(
                    out_ap=gt[:, 0:cfg.BB * CL, :], in_ap=tbl_h[0:cfg.SPLIT, :],
                    idxs_ap=s_hidxl[:, g * (nL // 16):(g + 1) * (nL // 16)],
                    num_idxs=nL, num_idxs_reg=nL, elem_size=ROWW,
                    single_packet=_SP, queue_num=(2 * g) % _NQ)
                nc.gpsimd.dma_gather(
                    out_ap=gt[:, cfg.BB * CL:JB, :], in_ap=tbl_h[cfg.SPLIT:NPAD, :],
                    idxs_ap=s_hidxh[:, g * (nH // 16):(g + 1) * (nH // 16)],
                    num_idxs=nH, num_idxs_reg=nH, elem_size=ROWW,
                    single_packet=_SP, queue_num=(2 * g + 1) % _NQ)
                # per-slot dst score: one-hot^T @ sd  (independent of gather)
                sdps = ps_sd.tile([128, JB, H], F32, name=f"sdps{layer}_{g}", tag="sdps")
                for c in range(JB):
                    nc.tensor.matmul(out=sdps[:, c, :], lhsT=osd[:, c, :],
                                     rhs=sdb[:, g, :], start=True, stop=True)
                # e = lrelu(ss + sd); alpha = exp(e)
                e_t = ep.tile([128, JB, H], F32, name=f"e{layer}_{g}", tag="e")
                nc.vector.tensor_tensor(out=e_t[:], in0=gt[:, :, HID:HID + H],
                                        in1=sdps[:], op=ALU.add)
                el = ep.tile([128, JB, H], F32, name=f"el{layer}_{g}", tag="el")
                nc.vector.tensor_scalar(out=el[:], in0=e_t[:], scalar1=cfg.neg_slope,
                                        scalar2=None, op0=ALU.mult)
                nc.vector.tensor_tensor(out=e_t[:], in0=e_t[:], in1=el[:], op=ALU.max)
                nc.scalar.activation(out=gt[:, :, HID:HID + H], in_=e_t[:], func=AF.Exp)
                # weight h by alpha, in two halves to overlap with aggregation
                for hf in range(2):
                    cs = slice(hf * (JB // 2), (hf + 1) * (JB // 2))
                    nc.vector.tensor_tensor(
                        out=gt[:, cs, 0:HID].rearrange("p a (h o) -> p a h o", o=OUT),
                        in0=gt[:, cs, 0:HID].rearrange("p a (h o) -> p a h o", o=OUT),
                        in1=gt[:, cs, HID:HID + H].to_broadcast([128, JB // 2, H, OUT]),
                        op=ALU.mult)
                for c in range(JB):
                    j = j0 + c
                    b, is_start, is_stop = cmap[j]
                    if is_start:
                        blk_psum[b] = ps_pair.tile(
                            [64, WA - H], F32, name=f"pp{layer}_{b}", tag="pp")
                    nc.tensor.matmul(
                        out=blk_psum[b][:], lhsT=oag[:, c, :],
                        rhs=gt[:, c, 0:WA - H],
                        start=is_start, stop=is_stop)
                    if is_stop and b % 2 == 1:
                        pair = b // 2
                        postproc(layer, pair, blk_psum[b - 1], blk_psum[b],
                                 bias_t, xT_next, ps_pool_t)
                        del blk_psum[b - 1], blk_psum[b]
                        if xT_next is not None:
                            table_chunk(layer + 1, pair, xT_next, w_next, 2,
                                        sdb_next)

        def postproc(layer, pair, ppz0, ppz1, bias_t, xT_next, ps_pool_t):
            den = pp.tile([128, H], F32, name=f"den{layer}_{pair}", tag="den")
            for hf, ppz in ((0, ppz0), (1, ppz1)):
                nc.vector.tensor_scalar(out=den[hf * 64:(hf + 1) * 64, :],
                                        in0=ppz[:, HID:HID + H],
                                        scalar1=1e-30, scalar2=None, op0=ALU.max)
            nc.vector.reciprocal(den[:], den[:])
            xf = pp.tile([128, HID], F32, name=f"xf{layer}_{pair}", tag="xf")
            for hf, ppz in ((0, ppz0), (1, ppz1)):
                nc.vector.tensor_tensor(
                    out=xf[hf * 64:(hf + 1) * 64, :].rearrange("p (h o) -> p h o", o=OUT),
                    in0=ppz[:, 0:HID].rearrange("p (h o) -> p h o", o=OUT),
                    in1=den[hf * 64:(hf + 1) * 64, :].to_broadcast([64, H, OUT]),
                    op=ALU.mult)
            nc.vector.tensor_tensor(out=xf[:], in0=xf[:], in1=bias_t[:], op=ALU.add)
            if layer == 0:
                xb = x1_res[:, pair, :]
            else:
                xb = pp.tile([128, HID], BF16, name=f"xb{layer}_{pair}", tag="xb")
            nc.scalar.activation(out=xb, in_=xf[:], func=AF.Relu)
            if layer < 2:
                for k in (0, 1):
                    pt = ps_t.tile([128, 128], BF16, name=f"pt{layer}_{pair}_{k}", tag="pt")
                    nc.tensor.transpose(out=pt[:], in_=xb[:, k * 128:(k + 1) * 128],
                                        identity=ident_bf[:])
                    nc.vector.tensor_copy(
                        out=xT_next[:, k, pair * 128:(pair + 1) * 128], in_=pt[:])
            else:
                xr = pp.tile([128, HID + 1], BF16, name=f"xr{pair}", tag="xr")
                nc.vector.memset(xr[:, HID:HID + 1], 1.0)
                nc.vector.tensor_tensor(out=xr[:, 0:HID], in0=xb,
                                        in1=x1_res[:, pair, :], op=ALU.add)
                nc.tensor.matmul(out=ps_pool_t[:], lhsT=s_og[:, pair, :],
                                 rhs=xr[:], start=(pair == 0),
                                 stop=(pair == PAIRS - 1))

        # ---------------- main flow ----------------
        ps_pool_t = ps_misc.tile([cfg.G, HID + 1], F32, tag="misc")

        sdb1 = wp.tile([128, NCHUNK, H], BF16, name="sdb1", tag="sdb")
        for i in range(NCHUNK):
            table_chunk(0, i, s_xT1, s_w1, 1, sdb1)
        nc.gpsimd.collective_compute(
            "AllGather", ALU.bypass, replica_groups=RG,
            ins=[hin_h[:].opt()], outs=[tbls_h[0][0:NPAD, :].opt()])

        xT2 = xtp.tile([128, 2, NLOC], BF16, name="xT2", tag="xt")
        sdb2 = wp.tile([128, NCHUNK, H], BF16, name="sdb2", tag="sdb")
        sdb_ref[0], sdb_ref[1] = sdb1, sdb2
        edge_phase(0, bias_ts[0], xT2, s_w2, None)
        nc.gpsimd.collective_compute(
            "AllGather", ALU.bypass, replica_groups=RG,
            ins=[hin_h[:].opt()], outs=[tbls_h[1][0:NPAD, :].opt()])

        xT3 = xtp.tile([128, 2, NLOC], BF16, name="xT3", tag="xt")
        sdb3 = wp.tile([128, NCHUNK, H], BF16, name="sdb3", tag="sdb")
        sdb_ref[0], sdb_ref[1] = sdb2, sdb3
        edge_phase(1, bias_ts[1], xT3, s_w3, None)
        nc.gpsimd.collective_compute(
            "AllGather", ALU.bypass, replica_groups=RG,
            ins=[hin_h[:].opt()], outs=[tbls_h[2][0:NPAD, :].opt()])

        sdb_ref[0], sdb_ref[1] = sdb3, None
        edge_phase(2, bias_ts[2], None, None, ps_pool_t)

        # ---------------- epilogue ----------------
        pl = sp.tile([cfg.G, HID + 1], F32)
        nc.vector.tensor_copy(pl[:], ps_pool_t[:])
        nc.sync.dma_start(out=pool_in[:, :], in_=pl[:])
        nc.gpsimd.collective_compute(
            "AllReduce", ALU.add, replica_groups=RG,
            ins=[pool_in[:].opt()], outs=[pool_out[:].opt()])
        pr = sp.tile([cfg.G, HID + 1], F32)
        nc.sync.dma_start(out=pr[:], in_=pool_out[:, :])
        cnt = sp.tile([cfg.G, 1], F32)
        nc.vector.tensor_scalar(out=cnt[:], in0=pr[:, HID:HID + 1],
                                scalar1=1.0, scalar2=None, op0=ALU.max)
        nc.vector.reciprocal(cnt[:], cnt[:])
        pa = sp.tile([cfg.G, HID + 1], F32)
        nc.vector.tensor_scalar(out=pa[:, 0:HID], in0=pr[:, 0:HID],
                                scalar1=cnt[:, 0:1], scalar2=None, op0=ALU.mult)
        nc.vector.memset(pa[:, HID:HID + 1], 1.0)
        paT = sp.tile([128, 2, cfg.G], F32)
        for k in (0, 1):
            pt = ps_t.tile([128, 128], F32, name=f"ptfc{k}", tag="pt")
            nc.tensor.transpose(out=pt[:, 0:cfg.G],
                                in_=pa[:, k * 128:(k + 1) * 128],
                                identity=ident_f32[0:cfg.G, 0:cfg.G])
            nc.vector.tensor_copy(paT[:, k, :], pt[:, 0:cfg.G])
        pfc = ps_misc.tile([cfg.G, cfg.C], F32, tag="misc")
        nc.tensor.matmul(out=pfc[:], lhsT=paT[:, 0, :], rhs=s_fcw[:, 0, :],
                         start=True, stop=False)
        nc.tensor.matmul(out=pfc[:], lhsT=paT[:, 1, :], rhs=s_fcw[:, 1, :],
                         start=False, stop=False)
        nc.tensor.matmul(out=pfc[:], lhsT=ones_row[:], rhs=s_fcw[0:1, 2, :],
                         start=False, stop=True)
        outt = sp.tile([cfg.G, cfg.C], F32)
        nc.vector.tensor_copy(outt[:], pfc[:])
        nc.sync.dma_start(out=d_out[:, :], in_=outt[:])

        for _pool in (ps_misc, ps_t, ps_sd, ps_pair, ps_h, sp, pp, ep, wp, xtp,
                      stage, consts, dram):
            _pool.release()

    nc.compile()
    return nc


# ----------------------------------------------------------------------------
# Entry point
# ----------------------------------------------------------------------------

_CACHE = {}


def _get_program(cfg):
    key = (cfg.N, cfg.BLOCKS, cfg.CPB_L, cfg.CPB_H, cfg.BB, cfg.NCORES,
           cfg.SPLIT_BLOCKS)
    if key not in _CACHE:
        _CACHE[key] = build_program(cfg)
    return _CACHE[key]


def kernel(x, edge_index, batch, W1, as1, ad1, b1, W2, as2, ad2, b2,
           W3, as3, ad3, b3, fcW, fcb):
    from concourse.bass_utils import run_bass_kernel_spmd
    cfg = Cfg()
    params = dict(W1=W1, as1=as1, ad1=ad1, b1=b1, W2=W2, as2=as2, ad2=ad2,
                  b2=b2, W3=W3, as3=as3, ad3=ad3, b3=b3, fcW=fcW, fcb=fcb)
    cfg, in_maps = preprocess(cfg, x, edge_index, batch, params)
    nc = _get_program(cfg)
    res = run_bass_kernel_spmd(nc, in_maps, core_ids=list(range(cfg.NCORES)))
    return np.asarray(res.results[0]["out"], dtype=np.float32)


# revision 8
# speedup vs baseline: 1.4578x; 1.0824x over previous
"""3-layer GAT (4 heads x 64) + global mean pool + FC on 8 Trainium2 NeuronCores.

Strategy (graph-parallel):
  * Nodes are permuted and partitioned into 8 contiguous shards (one per core),
    degree-balanced, and within each core greedily bin-packed into 100 blocks
    of 64 dst nodes balancing both low- and high-half in-degree.
  * Per layer, each core computes the augmented projection
    h_aug = x @ [W | W@As | W@Ad] for its own node shard; shards are
    AllGathered into a replicated DRAM table (row = 256 h bf16 + 4 ss + pad).
  * Edges live on the core that owns their dst node, sorted by (block, src
    half, dst).  The table is split at row SPLIT (3.5 cores ~ 44% of edge
    mass) so int16 gather indices reach both halves; per-block slot budget is
    4 low + 5 high 128-slot chunks.
  * Per 128-edge chunk the kernel gathers 768B table rows with SWDGE indirect
    DMAs round-robined over 4 queues (the per-queue descriptor feed is the
    bottleneck), adds the dst attention score via a one-hot matmul
    (host-precomputed dst-major one-hot x per-chunk sd from SBUF), applies
    leaky-relu+exp on the Activation engine, multiplies h by the per-edge
    weight, and contracts numerator+denominator into PSUM with a
    host-precomputed slot-major one-hot.
  * out = relu(num/den + b); residual after layer 3; the next layer's
    projection chunk is emitted inline right after each block pair finishes
    so only the AllGather sits between layers.  Graph mean-pool is a matmul
    against a host-built one-hot graph matrix + a tiny AllReduce; the final
    FC runs replicated on every core.
"""

import math
import os
import numpy as np

import concourse.bass as bass
import concourse.tile as tile
from concourse import bacc, mybir
from concourse.masks import make_identity

F32 = mybir.dt.float32
BF16 = mybir.dt.bfloat16
I32 = mybir.dt.int32
I16 = mybir.dt.int16
NPBF16 = mybir.dt.np(BF16)

AF = mybir.ActivationFunctionType
ALU = mybir.AluOpType
_NQ = int(os.environ.get('GAT_NQUEUES', '4'))
_SP = os.environ.get('GAT_SINGLE_PACKET', '0') == '1'


class Cfg:
    def __init__(self, N=50000, E=800000, IN=64, OUT=64, H=4, G=64, C=10,
                 NCORES=8, BLOCKS=100, BB=2, neg_slope=0.2,
                 CPB_L=4, CPB_H=5, SPLIT_BLOCKS=350):
        self.N, self.E, self.IN, self.OUT, self.H, self.G, self.C = N, E, IN, OUT, H, G, C
        self.HID = H * OUT                     # 256
        self.NCORES = NCORES
        self.BLOCKS = BLOCKS                   # dst blocks per core (64 nodes each)
        self.M = 64                            # dst nodes per block
        self.NLOC = BLOCKS * self.M            # nodes per core
        self.NPAD = NCORES * self.NLOC
        self.NCHUNK = self.NLOC // 128         # 128-node chunks per core
        self.PAIRS = self.NCHUNK
        self.CPB_L = CPB_L
        self.CPB_H = CPB_H
        self.CPBT = CPB_L + CPB_H              # chunks per block total
        self.BB = BB                           # blocks per batch (= node chunk)
        self.JB = BB * self.CPBT               # chunks per batch
        self.JTOT = BLOCKS * self.CPBT         # chunks per core
        self.NBATCH = BLOCKS // BB
        # table half boundary, in blocks across the whole table
        self.SPLIT_BLOCKS = SPLIT_BLOCKS
        self.SPLIT = SPLIT_BLOCKS * self.M
        self.neg_slope = neg_slope
        assert BLOCKS % 2 == 0 and BB == 2
        assert self.NLOC % 128 == 0
        assert self.SPLIT % self.NLOC in (0, self.NLOC // 2)  # core or half-core aligned
        assert self.SPLIT <= 32768 and self.NPAD - self.SPLIT <= 32768

    def chunk_map(self):
        "chunk j -> (block, start?, stop?) for the low/high grouped layout"
        out = []
        for g in range(self.NBATCH):
            for i in range(self.BB * self.CPB_L):
                b = g * self.BB + i // self.CPB_L
                out.append((b, i % self.CPB_L == 0, False))
            for i in range(self.BB * self.CPB_H):
                b = g * self.BB + i // self.CPB_H
                out.append((b, False, i % self.CPB_H == self.CPB_H - 1))
        return out


# ----------------------------------------------------------------------------
# Host-side preprocessing
# ----------------------------------------------------------------------------

def _snake(n, nbins):
    i = np.arange(n)
    m = i % (2 * nbins)
    return np.where(m < nbins, m, 2 * nbins - 1 - m)


def _pack_blocks(dlo, dhi, nblocks, cap_lo, cap_hi, M):
    """Greedily assign nodes (with per-node low/high in-degree) to nblocks
    blocks of at most M nodes, balancing both dims against the caps."""
    order = np.argsort(-(dlo + dhi), kind="stable")
    bl = np.zeros(nblocks)
    bh = np.zeros(nblocks)
    cnt = np.zeros(nblocks, np.int64)
    blk = np.empty(len(dlo), np.int64)
    for n in order:
        score = np.maximum((bl + dlo[n]) / cap_lo, (bh + dhi[n]) / cap_hi)
        score[cnt >= M] = np.inf
        b = int(np.argmin(score))
        blk[n] = b
        bl[b] += dlo[n]
        bh[b] += dhi[n]
        cnt[b] += 1
    return blk, int(bl.max()), int(bh.max())


def preprocess(cfg, x, edge_index, batch, params):
    N = cfg.N
    NC, B, NLOC, M = cfg.NCORES, cfg.BLOCKS, cfg.NLOC, cfg.M
    src0 = np.asarray(edge_index[0], dtype=np.int64)
    dst0 = np.asarray(edge_index[1], dtype=np.int64)
    batch = np.asarray(batch, dtype=np.int64)
    x = np.asarray(x, dtype=np.float32)

    deg = np.bincount(dst0, minlength=N) + 1          # + self loop
    order = np.argsort(-deg, kind="stable")           # nodes by in-degree desc

    # phase A: node -> core (degree-balanced snake)
    core_of_rank = _snake(N, NC)
    core_of = np.empty(N, dtype=np.int64)
    core_of[order] = core_of_rank

    # low/high classification: rows < SPLIT are "low".  SPLIT sits at
    # SPLIT_BLOCKS blocks; full cores below it are all-low, the boundary
    # core is split in half (its nodes snake-split into the two halves).
    split_core = cfg.SPLIT // NLOC                     # first (possibly) split core
    split_mid = (cfg.SPLIT % NLOC) != 0
    low_node = core_of < split_core
    half_lo_of = np.zeros(N, dtype=bool)               # for the split core only
    if split_mid:
        nodes_sc = order[core_of_rank == split_core]   # degree-desc order
        sel = (np.arange(len(nodes_sc)) % 2) == 0      # alternate halves
        half_lo_of[nodes_sc[sel]] = True
        low_node |= (core_of == split_core) & half_lo_of

    # per-node low/high in-degree
    dlow = np.bincount(dst0[low_node[src0]], minlength=N).astype(np.int64)
    dhigh = deg - 1 - dlow
    dlow = dlow + low_node
    dhigh = dhigh + (~low_node)

    # phase B: node -> block within core, 2D-balanced greedy packing
    CAP_L, CAP_H = cfg.CPB_L * 128, cfg.CPB_H * 128
    perm = np.empty(N, dtype=np.int64)
    maxlow = 0
    maxhigh = 0
    for c in range(NC):
        nodes_c = order[core_of_rank == c]
        if split_mid and c == split_core:
            parts = [(nodes_c[half_lo_of[nodes_c]], 0, B // 2),
                     (nodes_c[~half_lo_of[nodes_c]], B // 2, B // 2)]
        else:
            parts = [(nodes_c, 0, B)]
        for nodes_p, b0, nb in parts:
            assert len(nodes_p) <= nb * M
            blk, ml, mh = _pack_blocks(dlow[nodes_p], dhigh[nodes_p],
                                       nb, CAP_L, CAP_H, M)
            maxlow = max(maxlow, ml)
            maxhigh = max(maxhigh, mh)
            slot = np.zeros(len(nodes_p), dtype=np.int64)
            counts = np.zeros(nb, dtype=np.int64)
            for i in range(len(nodes_p)):
                b = blk[i]
                slot[i] = counts[b]
                counts[b] += 1
            perm[nodes_p] = c * NLOC + (b0 + blk) * M + slot

    cpb_l = max(cfg.CPB_L, math.ceil(maxlow / 128))
    cpb_h = max(cfg.CPB_H, math.ceil(maxhigh / 128))
    if (cpb_l, cpb_h) != (cfg.CPB_L, cfg.CPB_H):
        cfg = Cfg(N=cfg.N, E=cfg.E, IN=cfg.IN, OUT=cfg.OUT, H=cfg.H, G=cfg.G,
                  C=cfg.C, NCORES=cfg.NCORES, BLOCKS=cfg.BLOCKS, BB=cfg.BB,
                  neg_slope=cfg.neg_slope, CPB_L=cpb_l, CPB_H=cpb_h,
                  SPLIT_BLOCKS=cfg.SPLIT_BLOCKS)

    # ---- edge slot construction ----
    loops = np.arange(N, dtype=np.int64)
    srcE = np.concatenate([perm[src0], perm[loops]])
    dstE = np.concatenate([perm[dst0], perm[loops]])
    lowE = srcE < cfg.SPLIT
    blkE = dstE // M
    key = (blkE * 2 + (~lowE)) * (cfg.NPAD + 1) + dstE
    o = np.argsort(key, kind="stable")
    srcE, dstE, lowE = srcE[o], dstE[o], lowE[o]

    grp = blkE[o] * 2 + (~lowE)
    grp_counts = np.bincount(grp, minlength=NC * B * 2)
    grp_start = np.concatenate([[0], np.cumsum(grp_counts)])[:-1]
    rank = np.arange(len(dstE)) - grp_start[grp]

    CL, CH, CT = cfg.CPB_L, cfg.CPB_H, cfg.CPBT
    BBn = cfg.BB
    nslot_core = B * CT * 128
    core_e = dstE // NLOC
    b_in_core = (dstE % NLOC) // M
    g_e = b_in_core // BBn
    b_in_g = b_in_core % BBn
    group_base = core_e * nslot_core + g_e * (BBn * CT * 128)
    low_slot = group_base + b_in_g * (CL * 128) + rank
    high_slot = group_base + BBn * CL * 128 + b_in_g * (CH * 128) + rank
    slot = np.where(lowE, low_slot, high_slot)
    assert rank[lowE].max() < CL * 128 and rank[~lowE].max() < CH * 128

    nslots = NC * nslot_core
    hidx_slot = np.zeros(nslots, dtype=np.int16)
    dloc_slot = np.full(nslots, -1, dtype=np.int64)    # dst row within core
    hidx_slot[slot] = np.where(lowE, srcE, srcE - cfg.SPLIT).astype(np.int16)
    dloc_slot[slot] = dstE % NLOC

    JT = cfg.JTOT
    ngroups = B // BBn
    lowsel = np.zeros(nslot_core, dtype=bool)
    for g in range(ngroups):
        gb = g * BBn * CT * 128
        lowsel[gb:gb + BBn * CL * 128] = True

    # ---- weights ----
    W1, as1, ad1, b1 = params["W1"], params["as1"], params["ad1"], params["b1"]
    W2, as2, ad2, b2 = params["W2"], params["as2"], params["ad2"], params["b2"]
    W3, as3, ad3, b3 = params["W3"], params["as3"], params["ad3"], params["b3"]
    fcW, fcb = params["fcW"], params["fcb"]

    def aug(W, a_s, a_d):
        W = np.asarray(W, np.float32)
        HID, H, OUT = cfg.HID, cfg.H, cfg.OUT
        As = np.zeros((HID, H), np.float32)
        Ad = np.zeros((HID, H), np.float32)
        for h in range(H):
            As[h * OUT:(h + 1) * OUT, h] = np.asarray(a_s, np.float32)[h]
            Ad[h * OUT:(h + 1) * OUT, h] = np.asarray(a_d, np.float32)[h]
        return np.concatenate([W, W @ As, W @ Ad], axis=1)  # [in, HID+2H]

    w1a = aug(W1, as1, ad1)
    w2a = aug(W2, as2, ad2)
    w3a = aug(W3, as3, ad3)
    WA = cfg.HID + 2 * cfg.H                          # 264

    def pack_k(w):                                    # [256, WA] -> [128, 2*WA]
        return np.ascontiguousarray(
            w.reshape(2, 128, WA).transpose(1, 0, 2).reshape(128, 2 * WA))

    fcw_aug = np.concatenate([np.asarray(fcW, np.float32),
                              np.asarray(fcb, np.float32)[None, :]], axis=0)
    fcw_pad = np.zeros((384, cfg.C), np.float32)
    fcw_pad[:257] = fcw_aug
    fcw_m = np.ascontiguousarray(
        fcw_pad.reshape(3, 128, cfg.C).transpose(1, 0, 2).reshape(128, 3 * cfg.C))

    in_maps = []
    for c in range(NC):
        lo, hi = c * NLOC, (c + 1) * NLOC
        mask = (perm >= lo) & (perm < hi)
        origs = np.nonzero(mask)[0]
        locs = perm[origs] - lo
        xs = np.zeros((NLOC, cfg.IN), np.float32)
        xs[locs] = x[origs]
        og = np.zeros((NLOC, cfg.G), np.float32)
        og[locs, batch[origs]] = 1.0
        og_m = np.ascontiguousarray(
            og.reshape(cfg.NCHUNK, 128, cfg.G).transpose(1, 0, 2)
              .reshape(128, cfg.NCHUNK * cfg.G))
        hv = hidx_slot[c * nslot_core:(c + 1) * nslot_core]
        rep = lambda a: np.ascontiguousarray(np.tile(a.reshape(-1, 16).T, (8, 1)))
        # one-hots from the slot -> dst-row map
        dl = dloc_slot[c * nslot_core:(c + 1) * nslot_core]
        jj = np.arange(nslot_core) // 128              # chunk of each slot
        pp = np.arange(nslot_core) % 128               # partition of each slot
        valid = dl >= 0
        # slot-major [128, JTOT*128]: (p, j*128 + dst%128) — dst within pair
        o_ag = np.zeros((128, JT * 128), dtype=NPBF16)
        o_ag[pp[valid], jj[valid] * 128 + (dl[valid] % 128)] = 1.0
        # dst-major [128, JTOT*128]: (dst%128, j*128 + p)
        o_sd = np.zeros((128, JT * 128), dtype=NPBF16)
        o_sd[dl[valid] % 128, jj[valid] * 128 + pp[valid]] = 1.0
        in_maps.append({
            "xT": np.ascontiguousarray(xs.T).astype(NPBF16),
            "hidxl": rep(hv[lowsel]),
            "hidxh": rep(hv[~lowsel]),
            "oag": o_ag,
            "osd": o_sd,
            "og": og_m.astype(NPBF16),
            "w1": w1a.astype(NPBF16),
            "w2": pack_k(w2a).astype(NPBF16),
            "w3": pack_k(w3a).astype(NPBF16),
            "b1": np.asarray(b1, np.float32).reshape(1, cfg.HID),
            "b2": np.asarray(b2, np.float32).reshape(1, cfg.HID),
            "b3": np.asarray(b3, np.float32).reshape(1, cfg.HID),
            "fcw": fcw_m,
        })
    return cfg, in_maps


# ----------------------------------------------------------------------------
# Device program
# ----------------------------------------------------------------------------

def build_program(cfg, debug=False):
    nc = bacc.Bacc(None, target_bir_lowering=False, debug=debug,
                   num_devices=cfg.NCORES, num_swdge_queues=_NQ)
    HID, WA, H, OUT = cfg.HID, cfg.HID + 2 * cfg.H, cfg.H, cfg.OUT
    NLOC, NPAD, NCHUNK, PAIRS = cfg.NLOC, cfg.NPAD, cfg.NCHUNK, cfg.PAIRS
    JB, JTOT = cfg.JB, cfg.JTOT
    NBATCH = cfg.NBATCH
    CL, CH = cfg.CPB_L, cfg.CPB_H
    RG = [list(range(cfg.NCORES))]
    ROWW = 384  # padded table row (h 256 | ss 4 | sd 4 | pad)

    d_xT = nc.dram_tensor("xT", [cfg.IN, NLOC], BF16, kind="ExternalInput")
    d_hidxl = nc.dram_tensor("hidxl", [128, cfg.BLOCKS * CL * 8], I16, kind="ExternalInput")
    d_hidxh = nc.dram_tensor("hidxh", [128, cfg.BLOCKS * CH * 8], I16, kind="ExternalInput")
    d_oag = nc.dram_tensor("oag", [128, JTOT * 128], BF16, kind="ExternalInput")
    d_osd = nc.dram_tensor("osd", [128, JTOT * 128], BF16, kind="ExternalInput")
    d_og = nc.dram_tensor("og", [128, NCHUNK * cfg.G], BF16, kind="ExternalInput")
    d_w1 = nc.dram_tensor("w1", [cfg.IN, WA], BF16, kind="ExternalInput")
    d_w2 = nc.dram_tensor("w2", [128, 2 * WA], BF16, kind="ExternalInput")
    d_w3 = nc.dram_tensor("w3", [128, 2 * WA], BF16, kind="ExternalInput")
    d_b = [nc.dram_tensor(f"b{i}", [1, HID], F32, kind="ExternalInput")
           for i in (1, 2, 3)]
    d_fcw = nc.dram_tensor("fcw", [128, 3 * cfg.C], F32, kind="ExternalInput")
    d_out = nc.dram_tensor("out", [cfg.G, cfg.C], F32, kind="ExternalOutput")

    with tile.TileContext(nc, num_cores=cfg.NCORES) as tc:
        dram = tc.alloc_tile_pool(name="dram", bufs=1, space="DRAM")
        consts = tc.alloc_tile_pool(name="consts", bufs=1)
        stage = tc.alloc_tile_pool(name="stage", bufs=3)
        xtp = tc.alloc_tile_pool(name="xtp", bufs=1)
        wp = tc.alloc_tile_pool(name="wp", bufs=2)
        ep = tc.alloc_tile_pool(name="ep", bufs=3)
        pp = tc.alloc_tile_pool(name="pp", bufs=3)
        sp = tc.alloc_tile_pool(name="sp", bufs=2)
        ps_h = tc.alloc_tile_pool(name="ps_h", bufs=1, space="PSUM")
        ps_pair = tc.alloc_tile_pool(name="ps_pair", bufs=2, space="PSUM")
        ps_sd = tc.alloc_tile_pool(name="ps_sd", bufs=2, space="PSUM")
        ps_t = tc.alloc_tile_pool(name="ps_t", bufs=2, space="PSUM")
        ps_misc = tc.alloc_tile_pool(name="ps_misc", bufs=1, space="PSUM")

        # --- DRAM scratch ---
        hin_h = dram.tile([NLOC, ROWW], BF16)
        # one pad row: gathers of the last row read 768B from a 528B-used row
        tbls_h = [dram.tile([NPAD + 1, ROWW], BF16, addr_space="Shared",
                            name=f"tblh{i}") for i in range(3)]
        pool_in = dram.tile([cfg.G, HID + 1], F32)
        pool_out = dram.tile([cfg.G, HID + 1], F32, addr_space="Shared")

        # --- resident constants ---
        s_hidxl = consts.tile([128, cfg.BLOCKS * CL * 8], I16)
        s_hidxh = consts.tile([128, cfg.BLOCKS * CH * 8], I16)
        s_og = consts.tile([128, NCHUNK, cfg.G], BF16)
        nc.sync.dma_start(out=s_hidxl[:], in_=d_hidxl[:, :])
        nc.sync.dma_start(out=s_hidxh[:], in_=d_hidxh[:, :])
        nc.sync.dma_start(out=s_og[:], in_=d_og[:, :].rearrange("p (i g) -> p i g", g=cfg.G))

        s_xT1 = xtp.tile([cfg.IN, NLOC], BF16, tag="xt")
        nc.sync.dma_start(out=s_xT1[:], in_=d_xT[:, :])
        s_w1 = consts.tile([cfg.IN, WA], BF16)
        nc.sync.dma_start(out=s_w1[:], in_=d_w1[:, :])
        s_w2 = consts.tile([128, 2, WA], BF16)
        nc.sync.dma_start(out=s_w2[:], in_=d_w2[:, :].rearrange("p (k w) -> p k w", k=2))
        s_w3 = consts.tile([128, 2, WA], BF16)
        nc.sync.dma_start(out=s_w3[:], in_=d_w3[:, :].rearrange("p (k w) -> p k w", k=2))
        s_fcw = consts.tile([128, 3, cfg.C], F32)
        nc.sync.dma_start(out=s_fcw[:], in_=d_fcw[:, :].rearrange("p (k c) -> p k c", k=3))

        ident_bf = consts.tile([128, 128], BF16)
        make_identity(nc, ident_bf[:])
        ident_f32 = consts.tile([128, 128], F32)
        make_identity(nc, ident_f32[:])
        ones_row = consts.tile([1, cfg.G], F32)
        nc.vector.memset(ones_row[:], 1.0)

        x1_res = consts.tile([128, NCHUNK, HID], BF16)   # layer-1 activations

        bias_ts = []
        for i in range(3):
            bt = consts.tile([128, HID], F32, name=f"bias{i}", tag=f"bias{i}")
            nc.sync.dma_start(out=bt[:], in_=bass.AP(
                tensor=d_b[i][:, :].tensor, offset=0, ap=[[0, 128], [1, HID]]))
            bias_ts.append(bt)

        sdb_ref = [None, None]  # sdb tiles for current / next layer

        def table_chunk(layer, i, xT_t, w_t, khalves, sdb):
            """Emit projection of node-chunk i into table `layer` + sd capture."""
            ph = ps_h.tile([128, WA], F32, name=f"ph{layer}_{i}", tag="ph")
            for k in range(khalves):
                if khalves == 1:
                    lhsT = xT_t[:, i * 128:(i + 1) * 128]
                    rhs = w_t[:, :]
                else:
                    lhsT = xT_t[:, k, i * 128:(i + 1) * 128]
                    rhs = w_t[:, k, :]
                nc.tensor.matmul(out=ph[:], lhsT=lhsT, rhs=rhs,
                                 start=(k == 0), stop=(k == khalves - 1))
            h_st = stage.tile([128, HID + H], BF16, name=f"hst{layer}_{i}", tag="hst")
            nc.scalar.copy(out=h_st[:], in_=ph[:, 0:HID + H])
            nc.vector.tensor_copy(out=sdb[:, i, :], in_=ph[:, HID + H:WA])
            nc.sync.dma_start(
                out=hin_h[i * 128:(i + 1) * 128, 0:HID + H], in_=h_st[:])

        def edge_phase(layer, bias_t, xT_next, w_next, ps_pool_t):
            tbl_h = tbls_h[layer]
            sdb = sdb_ref[0]
            sdb_next = sdb_ref[1]
            nL, nH = cfg.BB * CL * 128, cfg.BB * CH * 128
            for g in range(NBATCH):
                j0 = g * JB
                # one-hot loads (independent of the table -> prefetch freely)
                oag = ep.tile([128, JB, 128], BF16, name=f"oag{layer}_{g}", tag="oag")
                nc.sync.dma_start(out=oag[:], in_=d_oag[:, j0 * 128:(j0 + JB) * 128]
                                  .rearrange("p (j m) -> p j m", m=128))
                osd = ep.tile([128, JB, 128], BF16, name=f"osd{layer}_{g}", tag="osd")
                nc.sync.dma_start(out=osd[:], in_=d_osd[:, j0 * 128:(j0 + JB) * 128]
                                  .rearrange("p (j m) -> p j m", m=128))
                gt = ep.tile([128, JB, ROWW], BF16, name=f"gt{layer}_{g}", tag="gt")
                nc.gpsimd.dma_gather(
                    out_ap=gt[:, 0:cfg.BB * CL, :], in_ap=tbl_h[0:cfg.SPLIT, :],
                    idxs_ap=s_hidxl[:, g * (nL // 16):(g + 1) * (nL // 16)],
                    num_idxs=nL, num_idxs_reg=nL, elem_size=ROWW,
                    single_packet=_SP, queue_num=(2 * g) % _NQ)
                nc.gpsimd.dma_gather(
                    out_ap=gt[:, cfg.BB * CL:JB, :], in_ap=tbl_h[cfg.SPLIT:NPAD, :],
                    idxs_ap=s_hidxh[:, g * (nH // 16):(g + 1) * (nH // 16)],
                    num_idxs=nH, num_idxs_reg=nH, elem_size=ROWW,
                    single_packet=_SP, queue_num=(2 * g + 1) % _NQ)
                # per-slot dst score: one-hot^T @ sd  (independent of gather)
                sdps = ps_sd.tile([128, JB, H], F32, name=f"sdps{layer}_{g}", tag="sdps")
                for c in range(JB):
                    nc.tensor.matmul(out=sdps[:, c, :], lhsT=osd[:, c, :],
                                     rhs=sdb[:, g, :], start=True, stop=True)
                # e = lrelu(ss + sd); alpha = exp(e)
                e_t = ep.tile([128, JB, H], F32, name=f"e{layer}_{g}", tag="e")
                nc.vector.tensor_tensor(out=e_t[:], in0=gt[:, :, HID:HID + H],
                                        in1=sdps[:], op=ALU.add)
                el = ep.tile([128, JB, H], F32, name=f"el{layer}_{g}", tag="el")
                nc.vector.tensor_scalar(out=el[:], in0=e_t[:], scalar1=cfg.neg_slope,
                                        scalar2=None, op0=ALU.mult)
                nc.vector.tensor_tensor(out=e_t[:], in0=e_t[:], in1=el[:], op=ALU.max)
                nc.scalar.activation(out=gt[:, :, HID:HID + H], in_=e_t[:], func=AF.Exp)
                # weight h by alpha
                nc.vector.tensor_tensor(
                    out=gt[:, :, 0:HID].rearrange("p a (h o) -> p a h o", o=OUT),
                    in0=gt[:, :, 0:HID].rearrange("p a (h o) -> p a h o", o=OUT),
                    in1=gt[:, :, HID:HID + H].to_broadcast([128, JB, H, OUT]),
                    op=ALU.mult)
                blk_ps = ps_pair.tile([128, WA - H], F32, name=f"pp{layer}_{g}",
                                      tag="pp")
                for c in range(JB):
                    nc.tensor.matmul(
                        out=blk_ps[:], lhsT=oag[:, c, :],
                        rhs=gt[:, c, 0:WA - H],
                        start=(c == 0), stop=(c == JB - 1))
                postproc(layer, g, blk_ps, bias_t, xT_next, ps_pool_t)
                if xT_next is not None:
                    table_chunk(layer + 1, g, xT_next, w_next, 2, sdb_next)

        def postproc(layer, pair, ppz, bias_t, xT_next, ps_pool_t):
            den = pp.tile([128, H], F32, name=f"den{layer}_{pair}", tag="den")
            nc.vector.tensor_scalar(out=den[:], in0=ppz[:, HID:HID + H],
                                    scalar1=1e-30, scalar2=None, op0=ALU.max)
            nc.vector.reciprocal(den[:], den[:])
            xf = pp.tile([128, HID], F32, name=f"xf{layer}_{pair}", tag="xf")
            nc.vector.tensor_tensor(
                out=xf[:].rearrange("p (h o) -> p h o", o=OUT),
                in0=ppz[:, 0:HID].rearrange("p (h o) -> p h o", o=OUT),
                in1=den[:].to_broadcast([128, H, OUT]),
                op=ALU.mult)
            nc.vector.tensor_tensor(out=xf[:], in0=xf[:], in1=bias_t[:], op=ALU.add)
            if layer == 0:
                xb = x1_res[:, pair, :]
            else:
                xb = pp.tile([128, HID], BF16, name=f"xb{layer}_{pair}", tag="xb")
            nc.scalar.activation(out=xb, in_=xf[:], func=AF.Relu)
            if layer < 2:
                for k in (0, 1):
                    pt = ps_t.tile([128, 128], BF16, name=f"pt{layer}_{pair}_{k}", tag="pt")
                    nc.tensor.transpose(out=pt[:], in_=xb[:, k * 128:(k + 1) * 128],
                                        identity=ident_bf[:])
                    nc.scalar.copy(
                        out=xT_next[:, k, pair * 128:(pair + 1) * 128], in_=pt[:])
            else:
                xr = pp.tile([128, HID + 1], BF16, name=f"xr{pair}", tag="xr")
                nc.vector.memset(xr[:, HID:HID + 1], 1.0)
                nc.vector.tensor_tensor(out=xr[:, 0:HID], in0=xb,
                                        in1=x1_res[:, pair, :], op=ALU.add)
                nc.tensor.matmul(out=ps_pool_t[:], lhsT=s_og[:, pair, :],
                                 rhs=xr[:], start=(pair == 0),
                                 stop=(pair == PAIRS - 1))

        # ---------------- main flow ----------------
        ps_pool_t = ps_misc.tile([cfg.G, HID + 1], F32, tag="misc")

        sdb1 = wp.tile([128, NCHUNK, H], BF16, name="sdb1", tag="sdb")
        for i in range(NCHUNK):
            table_chunk(0, i, s_xT1, s_w1, 1, sdb1)
        nc.gpsimd.collective_compute(
            "AllGather", ALU.bypass, replica_groups=RG,
            ins=[hin_h[:].opt()], outs=[tbls_h[0][0:NPAD, :].opt()])

        xT2 = xtp.tile([128, 2, NLOC], BF16, name="xT2", tag="xt")
        sdb2 = wp.tile([128, NCHUNK, H], BF16, name="sdb2", tag="sdb")
        sdb_ref[0], sdb_ref[1] = sdb1, sdb2
        edge_phase(0, bias_ts[0], xT2, s_w2, None)
        nc.gpsimd.collective_compute(
            "AllGather", ALU.bypass, replica_groups=RG,
            ins=[hin_h[:].opt()], outs=[tbls_h[1][0:NPAD, :].opt()])

        xT3 = xtp.tile([128, 2, NLOC], BF16, name="xT3", tag="xt")
        sdb3 = wp.tile([128, NCHUNK, H], BF16, name="sdb3", tag="sdb")
        sdb_ref[0], sdb_ref[1] = sdb2, sdb3
        edge_phase(1, bias_ts[1], xT3, s_w3, None)
        nc.gpsimd.collective_compute(
            "AllGather", ALU.bypass, replica_groups=RG,
            ins=[hin_h[:].opt()], outs=[tbls_h[2][0:NPAD, :].opt()])

        sdb_ref[0], sdb_ref[1] = sdb3, None
        edge_phase(2, bias_ts[2], None, None, ps_pool_t)

        # ---------------- epilogue ----------------
        pl = sp.tile([cfg.G, HID + 1], F32)
        nc.vector.tensor_copy(pl[:], ps_pool_t[:])
        nc.sync.dma_start(out=pool_in[:, :], in_=pl[:])
        nc.gpsimd.collective_compute(
            "AllReduce", ALU.add, replica_groups=RG,
            ins=[pool_in[:].opt()], outs=[pool_out[:].opt()])
        pr = sp.tile([cfg.G, HID + 1], F32)
        nc.sync.dma_start(out=pr[:], in_=pool_out[:, :])
        cnt = sp.tile([cfg.G, 1], F32)
        nc.vector.tensor_scalar(out=cnt[:], in0=pr[:, HID:HID + 1],
                                scalar1=1.0, scalar2=None, op0=ALU.max)
        nc.vector.reciprocal(cnt[:], cnt[:])
        pa = sp.tile([cfg.G, HID + 1], F32)
        nc.vector.tensor_scalar(out=pa[:, 0:HID], in0=pr[:, 0:HID],
                                scalar1=cnt[:, 0:1], scalar2=None, op0=ALU.mult)
        nc.vector.memset(pa[:, HID:HID + 1], 1.0)
        paT = sp.tile([128, 2, cfg.G], F32)
        for k in (0, 1):
            pt = ps_t.tile([128, 128], F32, name=f"ptfc{k}", tag="pt")
            nc.tensor.transpose(out=pt[:, 0:cfg.G],
                                in_=pa[:, k * 128:(k + 1) * 128],
                                identity=ident_f32[0:cfg.G, 0:cfg.G])
            nc.vector.tensor_copy(paT[:, k, :], pt[:, 0:cfg.G])
        pfc = ps_misc.tile([cfg.G, cfg.C], F32, tag="misc")
        nc.tensor.matmul(out=pfc[:], lhsT=paT[:, 0, :], rhs=s_fcw[:, 0, :],
                         start=True, stop=False)
        nc.tensor.matmul(out=pfc[:], lhsT=paT[:, 1, :], rhs=s_fcw[:, 1, :],
                         start=False, stop=False)
        nc.tensor.matmul(out=pfc[:], lhsT=ones_row[:], rhs=s_fcw[0:1, 2, :],
                         start=False, stop=True)
        outt = sp.tile([cfg.G, cfg.C], F32)
        nc.vector.tensor_copy(outt[:], pfc[:])
        nc.sync.dma_start(out=d_out[:, :], in_=outt[:])

        for _pool in (ps_misc, ps_t, ps_sd, ps_pair, ps_h, sp, pp, ep, wp, xtp,
                      stage, consts, dram):
            _pool.release()

    nc.compile()
    return nc


# ----------------------------------------------------------------------------
# Entry point
# ----------------------------------------------------------------------------

_CACHE = {}


def _get_program(cfg):
    key = (cfg.N, cfg.BLOCKS, cfg.CPB_L, cfg.CPB_H, cfg.BB, cfg.NCORES,
           cfg.SPLIT_BLOCKS)
    if key not in _CACHE:
        _CACHE[key] = build_program(cfg)
    return _CACHE[key]


def kernel(x, edge_index, batch, W1, as1, ad1, b1, W2, as2, ad2, b2,
           W3, as3, ad3, b3, fcW, fcb):
    from concourse.bass_utils import run_bass_kernel_spmd
    cfg = Cfg()
    params = dict(W1=W1, as1=as1, ad1=ad1, b1=b1, W2=W2, as2=as2, ad2=ad2,
                  b2=b2, W3=W3, as3=as3, ad3=ad3, b3=b3, fcW=fcW, fcb=fcb)
    cfg, in_maps = preprocess(cfg, x, edge_index, batch, params)
    nc = _get_program(cfg)
    res = run_bass_kernel_spmd(nc, in_maps, core_ids=list(range(cfg.NCORES)))
    return np.asarray(res.results[0]["out"], dtype=np.float32)


# revision 11
# speedup vs baseline: 1.4629x; 1.0035x over previous
"""3-layer GAT (4 heads x 64) + global mean pool + FC on 8 Trainium2 NeuronCores.

Strategy (graph-parallel):
  * Nodes are permuted and partitioned into 8 contiguous shards (one per core),
    degree-balanced, and within each core greedily bin-packed into 100 blocks
    of 64 dst nodes balancing both low- and high-half in-degree.
  * Per layer, each core computes the augmented projection
    h_aug = x @ [W | W@As | W@Ad] for its own node shard; shards are
    AllGathered into a replicated DRAM table (row = 256 h bf16 + 4 ss + pad).
  * Edges live on the core that owns their dst node, sorted by (block, src
    half, dst).  The table is split at row SPLIT (3.5 cores ~ 44% of edge
    mass) so int16 gather indices reach both halves; per-block slot budget is
    4 low + 5 high 128-slot chunks.
  * Per 128-edge chunk the kernel gathers 768B table rows with SWDGE indirect
    DMAs round-robined over 4 queues (the per-queue descriptor feed is the
    bottleneck), adds the dst attention score via a one-hot matmul
    (host-precomputed dst-major one-hot x per-chunk sd from SBUF), applies
    leaky-relu+exp on the Activation engine, multiplies h by the per-edge
    weight, and contracts numerator+denominator into PSUM with a
    host-precomputed slot-major one-hot.
  * out = relu(num/den + b); residual after layer 3; the next layer's
    projection chunk is emitted inline right after each block pair finishes
    so only the AllGather sits between layers.  Graph mean-pool is a matmul
    against a host-built one-hot graph matrix + a tiny AllReduce; the final
    FC runs replicated on every core.
"""

import math
import os
import numpy as np

import concourse.bass as bass
import concourse.tile as tile
from concourse import bacc, mybir
from concourse.masks import make_identity

F32 = mybir.dt.float32
BF16 = mybir.dt.bfloat16
I32 = mybir.dt.int32
I16 = mybir.dt.int16
NPBF16 = mybir.dt.np(BF16)

AF = mybir.ActivationFunctionType
ALU = mybir.AluOpType
_NQ = int(os.environ.get('GAT_NQUEUES', '4'))
_SP = os.environ.get('GAT_SINGLE_PACKET', '0') == '1'


class Cfg:
    def __init__(self, N=50000, E=800000, IN=64, OUT=64, H=4, G=64, C=10,
                 NCORES=8, BLOCKS=100, BB=2, neg_slope=0.2,
                 CPB_L=4, CPB_H=5, SPLIT_BLOCKS=350):
        self.N, self.E, self.IN, self.OUT, self.H, self.G, self.C = N, E, IN, OUT, H, G, C
        self.HID = H * OUT                     # 256
        self.NCORES = NCORES
        self.BLOCKS = BLOCKS                   # dst blocks per core (64 nodes each)
        self.M = 64                            # dst nodes per block
        self.NLOC = BLOCKS * self.M            # nodes per core
        self.NPAD = NCORES * self.NLOC
        self.NCHUNK = self.NLOC // 128         # 128-node chunks per core
        self.PAIRS = self.NCHUNK
        self.CPB_L = CPB_L
        self.CPB_H = CPB_H
        self.CPBT = CPB_L + CPB_H              # chunks per block total
        self.BB = BB                           # blocks per batch (= node chunk)
        self.JB = BB * self.CPBT               # chunks per batch
        self.JTOT = BLOCKS * self.CPBT         # chunks per core
        self.NBATCH = BLOCKS // BB
        # table half boundary, in blocks across the whole table
        self.SPLIT_BLOCKS = SPLIT_BLOCKS
        self.SPLIT = SPLIT_BLOCKS * self.M
        self.neg_slope = neg_slope
        assert BLOCKS % 2 == 0 and BB == 2
        assert self.NLOC % 128 == 0
        assert self.SPLIT % self.NLOC in (0, self.NLOC // 2)  # core or half-core aligned
        assert self.SPLIT <= 32768 and self.NPAD - self.SPLIT <= 32768

    def chunk_map(self):
        "chunk j -> (block, start?, stop?) for the low/high grouped layout"
        out = []
        for g in range(self.NBATCH):
            for i in range(self.BB * self.CPB_L):
                b = g * self.BB + i // self.CPB_L
                out.append((b, i % self.CPB_L == 0, False))
            for i in range(self.BB * self.CPB_H):
                b = g * self.BB + i // self.CPB_H
                out.append((b, False, i % self.CPB_H == self.CPB_H - 1))
        return out


# ----------------------------------------------------------------------------
# Host-side preprocessing
# ----------------------------------------------------------------------------

def _snake(n, nbins):
    i = np.arange(n)
    m = i % (2 * nbins)
    return np.where(m < nbins, m, 2 * nbins - 1 - m)


def _pack_blocks(dlo, dhi, nblocks, cap_lo, cap_hi, M):
    """Greedily assign nodes (with per-node low/high in-degree) to nblocks
    blocks of at most M nodes, balancing both dims against the caps."""
    order = np.argsort(-(dlo + dhi), kind="stable")
    bl = np.zeros(nblocks)
    bh = np.zeros(nblocks)
    cnt = np.zeros(nblocks, np.int64)
    blk = np.empty(len(dlo), np.int64)
    for n in order:
        score = np.maximum((bl + dlo[n]) / cap_lo, (bh + dhi[n]) / cap_hi)
        score[cnt >= M] = np.inf
        b = int(np.argmin(score))
        blk[n] = b
        bl[b] += dlo[n]
        bh[b] += dhi[n]
        cnt[b] += 1
    return blk, int(bl.max()), int(bh.max())


def preprocess(cfg, x, edge_index, batch, params):
    N = cfg.N
    NC, B, NLOC, M = cfg.NCORES, cfg.BLOCKS, cfg.NLOC, cfg.M
    src0 = np.asarray(edge_index[0], dtype=np.int64)
    dst0 = np.asarray(edge_index[1], dtype=np.int64)
    batch = np.asarray(batch, dtype=np.int64)
    x = np.asarray(x, dtype=np.float32)

    deg = np.bincount(dst0, minlength=N) + 1          # + self loop
    order = np.argsort(-deg, kind="stable")           # nodes by in-degree desc

    # phase A: node -> core (degree-balanced snake)
    core_of_rank = _snake(N, NC)
    core_of = np.empty(N, dtype=np.int64)
    core_of[order] = core_of_rank

    # low/high classification: rows < SPLIT are "low".  SPLIT sits at
    # SPLIT_BLOCKS blocks; full cores below it are all-low, the boundary
    # core is split in half (its nodes snake-split into the two halves).
    split_core = cfg.SPLIT // NLOC                     # first (possibly) split core
    split_mid = (cfg.SPLIT % NLOC) != 0
    low_node = core_of < split_core
    half_lo_of = np.zeros(N, dtype=bool)               # for the split core only
    if split_mid:
        nodes_sc = order[core_of_rank == split_core]   # degree-desc order
        sel = (np.arange(len(nodes_sc)) % 2) == 0      # alternate halves
        half_lo_of[nodes_sc[sel]] = True
        low_node |= (core_of == split_core) & half_lo_of

    # per-node low/high in-degree
    dlow = np.bincount(dst0[low_node[src0]], minlength=N).astype(np.int64)
    dhigh = deg - 1 - dlow
    dlow = dlow + low_node
    dhigh = dhigh + (~low_node)

    # phase B: node -> block within core, 2D-balanced greedy packing
    CAP_L, CAP_H = cfg.CPB_L * 128, cfg.CPB_H * 128
    perm = np.empty(N, dtype=np.int64)
    maxlow = 0
    maxhigh = 0
    for c in range(NC):
        nodes_c = order[core_of_rank == c]
        if split_mid and c == split_core:
            parts = [(nodes_c[half_lo_of[nodes_c]], 0, B // 2),
                     (nodes_c[~half_lo_of[nodes_c]], B // 2, B // 2)]
        else:
            parts = [(nodes_c, 0, B)]
        for nodes_p, b0, nb in parts:
            assert len(nodes_p) <= nb * M
            blk, ml, mh = _pack_blocks(dlow[nodes_p], dhigh[nodes_p],
                                       nb, CAP_L, CAP_H, M)
            maxlow = max(maxlow, ml)
            maxhigh = max(maxhigh, mh)
            slot = np.zeros(len(nodes_p), dtype=np.int64)
            counts = np.zeros(nb, dtype=np.int64)
            for i in range(len(nodes_p)):
                b = blk[i]
                slot[i] = counts[b]
                counts[b] += 1
            perm[nodes_p] = c * NLOC + (b0 + blk) * M + slot

    cpb_l = max(cfg.CPB_L, math.ceil(maxlow / 128))
    cpb_h = max(cfg.CPB_H, math.ceil(maxhigh / 128))
    if (cpb_l, cpb_h) != (cfg.CPB_L, cfg.CPB_H):
        cfg = Cfg(N=cfg.N, E=cfg.E, IN=cfg.IN, OUT=cfg.OUT, H=cfg.H, G=cfg.G,
                  C=cfg.C, NCORES=cfg.NCORES, BLOCKS=cfg.BLOCKS, BB=cfg.BB,
                  neg_slope=cfg.neg_slope, CPB_L=cpb_l, CPB_H=cpb_h,
                  SPLIT_BLOCKS=cfg.SPLIT_BLOCKS)

    # ---- edge slot construction ----
    loops = np.arange(N, dtype=np.int64)
    srcE = np.concatenate([perm[src0], perm[loops]])
    dstE = np.concatenate([perm[dst0], perm[loops]])
    lowE = srcE < cfg.SPLIT
    blkE = dstE // M
    key = (blkE * 2 + (~lowE)) * (cfg.NPAD + 1) + dstE
    o = np.argsort(key, kind="stable")
    srcE, dstE, lowE = srcE[o], dstE[o], lowE[o]

    grp = blkE[o] * 2 + (~lowE)
    grp_counts = np.bincount(grp, minlength=NC * B * 2)
    grp_start = np.concatenate([[0], np.cumsum(grp_counts)])[:-1]
    rank = np.arange(len(dstE)) - grp_start[grp]

    CL, CH, CT = cfg.CPB_L, cfg.CPB_H, cfg.CPBT
    BBn = cfg.BB
    nslot_core = B * CT * 128
    core_e = dstE // NLOC
    b_in_core = (dstE % NLOC) // M
    g_e = b_in_core // BBn
    b_in_g = b_in_core % BBn
    group_base = core_e * nslot_core + g_e * (BBn * CT * 128)
    low_slot = group_base + b_in_g * (CL * 128) + rank
    high_slot = group_base + BBn * CL * 128 + b_in_g * (CH * 128) + rank
    slot = np.where(lowE, low_slot, high_slot)
    assert rank[lowE].max() < CL * 128 and rank[~lowE].max() < CH * 128

    nslots = NC * nslot_core
    hidx_slot = np.zeros(nslots, dtype=np.int16)
    dloc_slot = np.full(nslots, -1, dtype=np.int64)    # dst row within core
    hidx_slot[slot] = np.where(lowE, srcE, srcE - cfg.SPLIT).astype(np.int16)
    dloc_slot[slot] = dstE % NLOC

    JT = cfg.JTOT
    ngroups = B // BBn
    lowsel = np.zeros(nslot_core, dtype=bool)
    for g in range(ngroups):
        gb = g * BBn * CT * 128
        lowsel[gb:gb + BBn * CL * 128] = True

    # ---- weights ----
    W1, as1, ad1, b1 = params["W1"], params["as1"], params["ad1"], params["b1"]
    W2, as2, ad2, b2 = params["W2"], params["as2"], params["ad2"], params["b2"]
    W3, as3, ad3, b3 = params["W3"], params["as3"], params["ad3"], params["b3"]
    fcW, fcb = params["fcW"], params["fcb"]

    def aug(W, a_s, a_d):
        W = np.asarray(W, np.float32)
        HID, H, OUT = cfg.HID, cfg.H, cfg.OUT
        As = np.zeros((HID, H), np.float32)
        Ad = np.zeros((HID, H), np.float32)
        for h in range(H):
            As[h * OUT:(h + 1) * OUT, h] = np.asarray(a_s, np.float32)[h]
            Ad[h * OUT:(h + 1) * OUT, h] = np.asarray(a_d, np.float32)[h]
        return np.concatenate([W, W @ As, W @ Ad], axis=1)  # [in, HID+2H]

    w1a = aug(W1, as1, ad1)
    w2a = aug(W2, as2, ad2)
    w3a = aug(W3, as3, ad3)
    WA = cfg.HID + 2 * cfg.H                          # 264

    def pack_k(w):                                    # [256, WA] -> [128, 2*WA]
        return np.ascontiguousarray(
            w.reshape(2, 128, WA).transpose(1, 0, 2).reshape(128, 2 * WA))

    fcw_aug = np.concatenate([np.asarray(fcW, np.float32),
                              np.asarray(fcb, np.float32)[None, :]], axis=0)
    fcw_pad = np.zeros((384, cfg.C), np.float32)
    fcw_pad[:257] = fcw_aug
    fcw_m = np.ascontiguousarray(
        fcw_pad.reshape(3, 128, cfg.C).transpose(1, 0, 2).reshape(128, 3 * cfg.C))

    in_maps = []
    for c in range(NC):
        lo, hi = c * NLOC, (c + 1) * NLOC
        mask = (perm >= lo) & (perm < hi)
        origs = np.nonzero(mask)[0]
        locs = perm[origs] - lo
        xs = np.zeros((NLOC, cfg.IN), np.float32)
        xs[locs] = x[origs]
        og = np.zeros((NLOC, cfg.G), np.float32)
        og[locs, batch[origs]] = 1.0
        og_m = np.ascontiguousarray(
            og.reshape(cfg.NCHUNK, 128, cfg.G).transpose(1, 0, 2)
              .reshape(128, cfg.NCHUNK * cfg.G))
        hv = hidx_slot[c * nslot_core:(c + 1) * nslot_core]
        rep = lambda a: np.ascontiguousarray(np.tile(a.reshape(-1, 16).T, (8, 1)))
        # one-hots from the slot -> dst-row map
        dl = dloc_slot[c * nslot_core:(c + 1) * nslot_core]
        jj = np.arange(nslot_core) // 128              # chunk of each slot
        pp = np.arange(nslot_core) % 128               # partition of each slot
        valid = dl >= 0
        # slot-major [128, JTOT*128]: (p, j*128 + dst%128) — dst within pair
        o_ag = np.zeros((128, JT * 128), dtype=NPBF16)
        o_ag[pp[valid], jj[valid] * 128 + (dl[valid] % 128)] = 1.0
        # dst-major [128, JTOT*128]: (dst%128, j*128 + p)
        o_sd = np.zeros((128, JT * 128), dtype=NPBF16)
        o_sd[dl[valid] % 128, jj[valid] * 128 + pp[valid]] = 1.0
        in_maps.append({
            "xT": np.ascontiguousarray(xs.T).astype(NPBF16),
            "hidxl": rep(hv[lowsel]),
            "hidxh": rep(hv[~lowsel]),
            "oag": o_ag,
            "osd": o_sd,
            "og": og_m.astype(NPBF16),
            "w1": w1a.astype(NPBF16),
            "w2": pack_k(w2a).astype(NPBF16),
            "w3": pack_k(w3a).astype(NPBF16),
            "b1": np.asarray(b1, np.float32).reshape(1, cfg.HID),
            "b2": np.asarray(b2, np.float32).reshape(1, cfg.HID),
            "b3": np.asarray(b3, np.float32).reshape(1, cfg.HID),
            "fcw": fcw_m,
        })
    return cfg, in_maps


# ----------------------------------------------------------------------------
# Device program
# ----------------------------------------------------------------------------

def build_program(cfg, debug=False):
    nc = bacc.Bacc(None, target_bir_lowering=False, debug=debug,
                   num_devices=cfg.NCORES, num_swdge_queues=_NQ)
    HID, WA, H, OUT = cfg.HID, cfg.HID + 2 * cfg.H, cfg.H, cfg.OUT
    NLOC, NPAD, NCHUNK, PAIRS = cfg.NLOC, cfg.NPAD, cfg.NCHUNK, cfg.PAIRS
    JB, JTOT = cfg.JB, cfg.JTOT
    NBATCH = cfg.NBATCH
    CL, CH = cfg.CPB_L, cfg.CPB_H
    RG = [list(range(cfg.NCORES))]
    ROWW = 384  # padded table row (h 256 | ss 4 | sd 4 | pad)

    d_xT = nc.dram_tensor("xT", [cfg.IN, NLOC], BF16, kind="ExternalInput")
    d_hidxl = nc.dram_tensor("hidxl", [128, cfg.BLOCKS * CL * 8], I16, kind="ExternalInput")
    d_hidxh = nc.dram_tensor("hidxh", [128, cfg.BLOCKS * CH * 8], I16, kind="ExternalInput")
    d_oag = nc.dram_tensor("oag", [128, JTOT * 128], BF16, kind="ExternalInput")
    d_osd = nc.dram_tensor("osd", [128, JTOT * 128], BF16, kind="ExternalInput")
    d_og = nc.dram_tensor("og", [128, NCHUNK * cfg.G], BF16, kind="ExternalInput")
    d_w1 = nc.dram_tensor("w1", [cfg.IN, WA], BF16, kind="ExternalInput")
    d_w2 = nc.dram_tensor("w2", [128, 2 * WA], BF16, kind="ExternalInput")
    d_w3 = nc.dram_tensor("w3", [128, 2 * WA], BF16, kind="ExternalInput")
    d_b = [nc.dram_tensor(f"b{i}", [1, HID], F32, kind="ExternalInput")
           for i in (1, 2, 3)]
    d_fcw = nc.dram_tensor("fcw", [128, 3 * cfg.C], F32, kind="ExternalInput")
    d_out = nc.dram_tensor("out", [cfg.G, cfg.C], F32, kind="ExternalOutput")

    with tile.TileContext(nc, num_cores=cfg.NCORES) as tc:
        dram = tc.alloc_tile_pool(name="dram", bufs=1, space="DRAM")
        consts = tc.alloc_tile_pool(name="consts", bufs=1)
        stage = tc.alloc_tile_pool(name="stage", bufs=3)
        xtp = tc.alloc_tile_pool(name="xtp", bufs=1)
        wp = tc.alloc_tile_pool(name="wp", bufs=2)
        ep = tc.alloc_tile_pool(name="ep", bufs=3)
        pp = tc.alloc_tile_pool(name="pp", bufs=3)
        sp = tc.alloc_tile_pool(name="sp", bufs=2)
        ps_h = tc.alloc_tile_pool(name="ps_h", bufs=1, space="PSUM")
        ps_pair = tc.alloc_tile_pool(name="ps_pair", bufs=2, space="PSUM")
        ps_sd = tc.alloc_tile_pool(name="ps_sd", bufs=2, space="PSUM")
        ps_t = tc.alloc_tile_pool(name="ps_t", bufs=2, space="PSUM")
        ps_misc = tc.alloc_tile_pool(name="ps_misc", bufs=1, space="PSUM")

        # --- DRAM scratch ---
        hin_h = dram.tile([NLOC, ROWW], BF16)
        # one pad row: gathers of the last row read 768B from a 528B-used row
        tbls_h = [dram.tile([NPAD + 1, ROWW], BF16, addr_space="Shared",
                            name=f"tblh{i}") for i in range(3)]
        pool_in = dram.tile([cfg.G, HID + 1], F32)
        pool_out = dram.tile([cfg.G, HID + 1], F32, addr_space="Shared")

        # --- resident constants ---
        s_hidxl = consts.tile([128, cfg.BLOCKS * CL * 8], I16)
        s_hidxh = consts.tile([128, cfg.BLOCKS * CH * 8], I16)
        s_og = consts.tile([128, NCHUNK, cfg.G], BF16)
        nc.sync.dma_start(out=s_hidxl[:], in_=d_hidxl[:, :])
        nc.sync.dma_start(out=s_hidxh[:], in_=d_hidxh[:, :])
        nc.sync.dma_start(out=s_og[:], in_=d_og[:, :].rearrange("p (i g) -> p i g", g=cfg.G))

        s_xT1 = xtp.tile([cfg.IN, NLOC], BF16, tag="xt")
        nc.sync.dma_start(out=s_xT1[:], in_=d_xT[:, :])
        s_w1 = consts.tile([cfg.IN, WA], BF16)
        nc.sync.dma_start(out=s_w1[:], in_=d_w1[:, :])
        s_w2 = consts.tile([128, 2, WA], BF16)
        nc.sync.dma_start(out=s_w2[:], in_=d_w2[:, :].rearrange("p (k w) -> p k w", k=2))
        s_w3 = consts.tile([128, 2, WA], BF16)
        nc.sync.dma_start(out=s_w3[:], in_=d_w3[:, :].rearrange("p (k w) -> p k w", k=2))
        s_fcw = consts.tile([128, 3, cfg.C], F32)
        nc.sync.dma_start(out=s_fcw[:], in_=d_fcw[:, :].rearrange("p (k c) -> p k c", k=3))

        ident_bf = consts.tile([128, 128], BF16)
        make_identity(nc, ident_bf[:])
        ident_f32 = consts.tile([128, 128], F32)
        make_identity(nc, ident_f32[:])
        ones_row = consts.tile([1, cfg.G], F32)
        nc.vector.memset(ones_row[:], 1.0)

        x1_res = consts.tile([128, NCHUNK, HID], BF16)   # layer-1 activations

        bias_ts = []
        for i in range(3):
            bt = consts.tile([128, HID], F32, name=f"bias{i}", tag=f"bias{i}")
            nc.sync.dma_start(out=bt[:], in_=bass.AP(
                tensor=d_b[i][:, :].tensor, offset=0, ap=[[0, 128], [1, HID]]))
            bias_ts.append(bt)

        sdb_ref = [None, None]  # sdb tiles for current / next layer

        def table_chunk(layer, i, xT_t, w_t, khalves, sdb):
            """Emit projection of node-chunk i into table `layer` + sd capture."""
            ph = ps_h.tile([128, WA], F32, name=f"ph{layer}_{i}", tag="ph")
            for k in range(khalves):
                if khalves == 1:
                    lhsT = xT_t[:, i * 128:(i + 1) * 128]
                    rhs = w_t[:, :]
                else:
                    lhsT = xT_t[:, k, i * 128:(i + 1) * 128]
                    rhs = w_t[:, k, :]
                nc.tensor.matmul(out=ph[:], lhsT=lhsT, rhs=rhs,
                                 start=(k == 0), stop=(k == khalves - 1))
            h_st = stage.tile([128, HID + H], BF16, name=f"hst{layer}_{i}", tag="hst")
            nc.scalar.copy(out=h_st[:], in_=ph[:, 0:HID + H])
            nc.vector.tensor_copy(out=sdb[:, i, :], in_=ph[:, HID + H:WA])
            nc.sync.dma_start(
                out=hin_h[i * 128:(i + 1) * 128, 0:HID + H], in_=h_st[:])

        def edge_phase(layer, bias_t, xT_next, w_next, ps_pool_t):
            tbl_h = tbls_h[layer]
            sdb = sdb_ref[0]
            sdb_next = sdb_ref[1]
            nL, nH = cfg.BB * CL * 128, cfg.BB * CH * 128
            for g in range(NBATCH):
                j0 = g * JB
                # one-hot loads (independent of the table -> prefetch freely)
                oag = ep.tile([128, JB, 128], BF16, name=f"oag{layer}_{g}", tag="oag")
                nc.sync.dma_start(out=oag[:], in_=d_oag[:, j0 * 128:(j0 + JB) * 128]
                                  .rearrange("p (j m) -> p j m", m=128))
                osd = ep.tile([128, JB, 128], BF16, name=f"osd{layer}_{g}", tag="osd")
                nc.sync.dma_start(out=osd[:], in_=d_osd[:, j0 * 128:(j0 + JB) * 128]
                                  .rearrange("p (j m) -> p j m", m=128))
                gt = ep.tile([128, JB, ROWW], BF16, name=f"gt{layer}_{g}", tag="gt",
                             bufs=4)
                nc.gpsimd.dma_gather(
                    out_ap=gt[:, 0:cfg.BB * CL, :], in_ap=tbl_h[0:cfg.SPLIT, :],
                    idxs_ap=s_hidxl[:, g * (nL // 16):(g + 1) * (nL // 16)],
                    num_idxs=nL, num_idxs_reg=nL, elem_size=ROWW,
                    single_packet=_SP, queue_num=(2 * g) % _NQ)
                nc.gpsimd.dma_gather(
                    out_ap=gt[:, cfg.BB * CL:JB, :], in_ap=tbl_h[cfg.SPLIT:NPAD, :],
                    idxs_ap=s_hidxh[:, g * (nH // 16):(g + 1) * (nH // 16)],
                    num_idxs=nH, num_idxs_reg=nH, elem_size=ROWW,
                    single_packet=_SP, queue_num=(2 * g + 1) % _NQ)
                # per-slot dst score: one-hot^T @ sd  (independent of gather)
                sdps = ps_sd.tile([128, JB, H], F32, name=f"sdps{layer}_{g}", tag="sdps")
                for c in range(JB):
                    nc.tensor.matmul(out=sdps[:, c, :], lhsT=osd[:, c, :],
                                     rhs=sdb[:, g, :], start=True, stop=True)
                # e = lrelu(ss + sd); alpha = exp(e)
                # e = lrelu(ss+sd) via one fused op: (ss*slope) max (ss+sd)...
                # lrelu(x) = max(x, slope*x); fuse as (in0*slope) max in1 where
                # in0 = in1 = x is not expressible, so: tmp = x*slope (vector),
                # e = max(x, tmp).  Use scalar_tensor_tensor to fuse both:
                # out = (in0 mult slope) max in1 with in0 = in1 = x.
                e_t = ep.tile([128, JB, H], F32, name=f"e{layer}_{g}", tag="e")
                nc.vector.tensor_tensor(out=e_t[:], in0=gt[:, :, HID:HID + H],
                                        in1=sdps[:], op=ALU.add)
                el = ep.tile([128, JB, H], F32, name=f"el{layer}_{g}", tag="el")
                nc.vector.scalar_tensor_tensor(
                    out=el[:], in0=e_t[:], scalar=cfg.neg_slope, in1=e_t[:],
                    op0=ALU.mult, op1=ALU.max)
                nc.scalar.activation(out=gt[:, :, HID:HID + H], in_=el[:], func=AF.Exp)
                # weight h by alpha
                nc.vector.tensor_tensor(
                    out=gt[:, :, 0:HID].rearrange("p a (h o) -> p a h o", o=OUT),
                    in0=gt[:, :, 0:HID].rearrange("p a (h o) -> p a h o", o=OUT),
                    in1=gt[:, :, HID:HID + H].to_broadcast([128, JB, H, OUT]),
                    op=ALU.mult)
                blk_ps = ps_pair.tile([128, WA - H], F32, name=f"pp{layer}_{g}",
                                      tag="pp")
                for c in range(JB):
                    nc.tensor.matmul(
                        out=blk_ps[:], lhsT=oag[:, c, :],
                        rhs=gt[:, c, 0:WA - H],
                        start=(c == 0), stop=(c == JB - 1))
                postproc(layer, g, blk_ps, bias_t, xT_next, ps_pool_t)
                if xT_next is not None:
                    table_chunk(layer + 1, g, xT_next, w_next, 2, sdb_next)

        def postproc(layer, pair, ppz, bias_t, xT_next, ps_pool_t):
            den = pp.tile([128, H], F32, name=f"den{layer}_{pair}", tag="den")
            nc.vector.tensor_scalar(out=den[:], in0=ppz[:, HID:HID + H],
                                    scalar1=1e-30, scalar2=None, op0=ALU.max)
            nc.vector.reciprocal(den[:], den[:])
            xf = pp.tile([128, HID], F32, name=f"xf{layer}_{pair}", tag="xf")
            nc.vector.tensor_tensor(
                out=xf[:].rearrange("p (h o) -> p h o", o=OUT),
                in0=ppz[:, 0:HID].rearrange("p (h o) -> p h o", o=OUT),
                in1=den[:].to_broadcast([128, H, OUT]),
                op=ALU.mult)
            nc.vector.tensor_tensor(out=xf[:], in0=xf[:], in1=bias_t[:], op=ALU.add)
            if layer == 0:
                xb = x1_res[:, pair, :]
            else:
                xb = pp.tile([128, HID], BF16, name=f"xb{layer}_{pair}", tag="xb")
            nc.scalar.activation(out=xb, in_=xf[:], func=AF.Relu)
            if layer < 2:
                for k in (0, 1):
                    pt = ps_t.tile([128, 128], BF16, name=f"pt{layer}_{pair}_{k}", tag="pt")
                    nc.tensor.transpose(out=pt[:], in_=xb[:, k * 128:(k + 1) * 128],
                                        identity=ident_bf[:])
                    nc.scalar.copy(
                        out=xT_next[:, k, pair * 128:(pair + 1) * 128], in_=pt[:])
            else:
                xr = pp.tile([128, HID + 1], BF16, name=f"xr{pair}", tag="xr")
                nc.vector.memset(xr[:, HID:HID + 1], 1.0)
                nc.vector.tensor_tensor(out=xr[:, 0:HID], in0=xb,
                                        in1=x1_res[:, pair, :], op=ALU.add)
                nc.tensor.matmul(out=ps_pool_t[:], lhsT=s_og[:, pair, :],
                                 rhs=xr[:], start=(pair == 0),
                                 stop=(pair == PAIRS - 1))

        # ---------------- main flow ----------------
        ps_pool_t = ps_misc.tile([cfg.G, HID + 1], F32, tag="misc")

        sdb1 = wp.tile([128, NCHUNK, H], BF16, name="sdb1", tag="sdb")
        for i in range(NCHUNK):
            table_chunk(0, i, s_xT1, s_w1, 1, sdb1)
        nc.gpsimd.collective_compute(
            "AllGather", ALU.bypass, replica_groups=RG,
            ins=[hin_h[:].opt()], outs=[tbls_h[0][0:NPAD, :].opt()])

        xT2 = xtp.tile([128, 2, NLOC], BF16, name="xT2", tag="xt")
        sdb2 = wp.tile([128, NCHUNK, H], BF16, name="sdb2", tag="sdb")
        sdb_ref[0], sdb_ref[1] = sdb1, sdb2
        edge_phase(0, bias_ts[0], xT2, s_w2, None)
        nc.gpsimd.collective_compute(
            "AllGather", ALU.bypass, replica_groups=RG,
            ins=[hin_h[:].opt()], outs=[tbls_h[1][0:NPAD, :].opt()])

        xT3 = xtp.tile([128, 2, NLOC], BF16, name="xT3", tag="xt")
        sdb3 = wp.tile([128, NCHUNK, H], BF16, name="sdb3", tag="sdb")
        sdb_ref[0], sdb_ref[1] = sdb2, sdb3
        edge_phase(1, bias_ts[1], xT3, s_w3, None)
        nc.gpsimd.collective_compute(
            "AllGather", ALU.bypass, replica_groups=RG,
            ins=[hin_h[:].opt()], outs=[tbls_h[2][0:NPAD, :].opt()])

        sdb_ref[0], sdb_ref[1] = sdb3, None
        edge_phase(2, bias_ts[2], None, None, ps_pool_t)

        # ---------------- epilogue ----------------
        pl = sp.tile([cfg.G, HID + 1], F32)
        nc.vector.tensor_copy(pl[:], ps_pool_t[:])
        nc.sync.dma_start(out=pool_in[:, :], in_=pl[:])
        nc.gpsimd.collective_compute(
            "AllReduce", ALU.add, replica_groups=RG,
            ins=[pool_in[:].opt()], outs=[pool_out[:].opt()])
        pr = sp.tile([cfg.G, HID + 1], F32)
        nc.sync.dma_start(out=pr[:], in_=pool_out[:, :])
        cnt = sp.tile([cfg.G, 1], F32)
        nc.vector.tensor_scalar(out=cnt[:], in0=pr[:, HID:HID + 1],
                                scalar1=1.0, scalar2=None, op0=ALU.max)
        nc.vector.reciprocal(cnt[:], cnt[:])
        pa = sp.tile([cfg.G, HID + 1], F32)
        nc.vector.tensor_scalar(out=pa[:, 0:HID], in0=pr[:, 0:HID],
                                scalar1=cnt[:, 0:1], scalar2=None, op0=ALU.mult)
        nc.vector.memset(pa[:, HID:HID + 1], 1.0)
        paT = sp.tile([128, 2, cfg.G], F32)
        for k in (0, 1):
            pt = ps_t.tile([128, 128], F32, name=f"ptfc{k}", tag="pt")
            nc.tensor.transpose(out=pt[:, 0:cfg.G],
                                in_=pa[:, k * 128:(k + 1) * 128],
                                identity=ident_f32[0:cfg.G, 0:cfg.G])
            nc.vector.tensor_copy(paT[:, k, :], pt[:, 0:cfg.G])
        pfc = ps_misc.tile([cfg.G, cfg.C], F32, tag="misc")
        nc.tensor.matmul(out=pfc[:], lhsT=paT[:, 0, :], rhs=s_fcw[:, 0, :],
                         start=True, stop=False)
        nc.tensor.matmul(out=pfc[:], lhsT=paT[:, 1, :], rhs=s_fcw[:, 1, :],
                         start=False, stop=False)
        nc.tensor.matmul(out=pfc[:], lhsT=ones_row[:], rhs=s_fcw[0:1, 2, :],
                         start=False, stop=True)
        outt = sp.tile([cfg.G, cfg.C], F32)
        nc.vector.tensor_copy(outt[:], pfc[:])
        nc.sync.dma_start(out=d_out[:, :], in_=outt[:])

        for _pool in (ps_misc, ps_t, ps_sd, ps_pair, ps_h, sp, pp, ep, wp, xtp,
                      stage, consts, dram):
            _pool.release()

    nc.compile()
    return nc


# ----------------------------------------------------------------------------
# Entry point
# ----------------------------------------------------------------------------

_CACHE = {}


def _get_program(cfg):
    key = (cfg.N, cfg.BLOCKS, cfg.CPB_L, cfg.CPB_H, cfg.BB, cfg.NCORES,
           cfg.SPLIT_BLOCKS)
    if key not in _CACHE:
        _CACHE[key] = build_program(cfg)
    return _CACHE[key]


def kernel(x, edge_index, batch, W1, as1, ad1, b1, W2, as2, ad2, b2,
           W3, as3, ad3, b3, fcW, fcb):
    from concourse.bass_utils import run_bass_kernel_spmd
    cfg = Cfg()
    params = dict(W1=W1, as1=as1, ad1=ad1, b1=b1, W2=W2, as2=as2, ad2=ad2,
                  b2=b2, W3=W3, as3=as3, ad3=ad3, b3=b3, fcW=fcW, fcb=fcb)
    cfg, in_maps = preprocess(cfg, x, edge_index, batch, params)
    nc = _get_program(cfg)
    res = run_bass_kernel_spmd(nc, in_maps, core_ids=list(range(cfg.NCORES)))
    return np.asarray(res.results[0]["out"], dtype=np.float32)
